# revision 2
# baseline (speedup 1.0000x reference)
"""Trainium2 Bass kernel for MultiLayer bidirectional BTreeLSTM (4096-node
balanced heap tree, IN=OUT=1024, H=512, L=2).

v2 strategy (vs v1 baseline):
- x2/px projections stay SBUF-resident in bf16 (no DRAM round-trip), one
  weight pass per layer with two 260-col matmul chunks per LDWEIGHTS.
- Scan matmuls read h directly from the persistent FB tiles via strided
  APs (no staging copies); the backward direction multiplies over the
  m/2 parent columns only (both children share W@h_parent) and the
  per-child x2 add re-expands via strided adds.
- Gate blocks are reordered host-side to [i,o,fl,fr,r,u] (fw) and
  [i,o,f,r,u] (bw) so small levels (m<=16) run as one PSUM tile with a
  single gate-sum add and just two activation instructions.
- Small-level outputs accumulate in an f32 SBUF strip (cols 0..133) and
  ship to DRAM in one DMA per 128-row block.

Per-core column layout (NC=520): [0..6]=nodes 0..6, [7..517]=subtree
levels 3..11 in level order, [518]=level-12 slot (node 4095, core 0 only),
[519]=pad.
"""

import numpy as np
import ml_dtypes

F8NP = ml_dtypes.float8_e4m3fn
N = 4096
H = 512
L = 2
NCORES = 8
NC = 520
BF16NP = ml_dtypes.bfloat16

_CACHE = {}

# gate-block permutations (rows of the 6H/5H gate dim)
_PF = np.concatenate([np.arange(512 * b, 512 * (b + 1))
                      for b in (0, 1, 2, 3, 5, 4)])  # i,o,fl,fr,r,u
_PB = np.concatenate([np.arange(512 * b, 512 * (b + 1))
                      for b in (0, 1, 2, 4, 3)])     # i,o,f,r,u


# ----------------------------------------------------------------- host utils
def _lvl_off(lvl):
    return 7 + (1 << (lvl - 3)) - 1


def _col_map(core):
    r = 7 + core
    ids = list(range(7))
    for lvl in range(3, 12):
        w = 1 << (lvl - 3)
        start = (r + 1) * w - 1
        ids.extend(range(start, start + w))
    ids.append(4095 if core == 0 else -1)
    ids.append(-1)
    return np.array(ids, np.int64)


def _is_canonical(inp):
    n = N
    i = np.arange(n)
    left = np.where(2 * i + 1 < n, 2 * i + 1, n).astype(np.int32)
    right = np.where(2 * i + 2 < n, 2 * i + 2, n).astype(np.int32)
    parent = np.where(i > 0, (i - 1) // 2, n).astype(np.int32)
    if inp["features"].shape != (N, 1024):
        return False
    for k, v in (("left_child", left), ("right_child", right), ("parent", parent)):
        if inp[k].shape != (n,) or not np.array_equal(np.asarray(inp[k]), v):
            return False
    po = np.asarray(inp["post_order"])
    pr = np.asarray(inp["pre_order"])
    if sorted(po.tolist()) != list(range(n)) or sorted(pr.tolist()) != list(range(n)):
        return False
    pos = np.empty(n, np.int64)
    pos[po] = np.arange(n)
    ok = True
    for child in (left, right):
        m = child < n
        ok &= bool((pos[i[m]] > pos[child[m]]).all())
    pos[pr] = np.arange(n)
    m = parent < n
    ok &= bool((pos[i[m]] > pos[parent[m]]).all())
    return ok


def _fallback(inp):
    """Literal numpy re-implementation of the reference scan (any inputs)."""
    f = {k: np.asarray(v) for k, v in inp.items()}
    feats = f["features"].astype(np.float32)
    n = feats.shape[0]

    def sig(x):
        return 1.0 / (1.0 + np.exp(-x))

    for l in range(L):
        h = f["fw_bp"][l].shape[0]
        px = feats @ f["fw_Wp"][l].T + f["fw_bp"][l]
        x2 = feats @ f["fw_Wx"][l].T + f["fw_bx"][l]
        cbuf = np.zeros((n + 1, h), np.float32)
        hbuf = np.zeros((n + 1, h), np.float32)
        Wl, bl, Wr, br = f["fw_Wl"][l], f["fw_bl"][l], f["fw_Wr"][l], f["fw_br"][l]
        for idx in f["post_order"]:
            lc, rc = f["left_child"][idx], f["right_child"][idx]
            g = x2[idx] + hbuf[lc] @ Wl.T + bl + hbuf[rc] @ Wr.T + br
            i_, o, fl, fr, u, r = np.split(g, 6)
            i_, o, fl, fr, r = sig(i_), sig(o), sig(fl), sig(fr), sig(r)
            u = np.tanh(u)
            c = i_ * u + fl * cbuf[lc] + fr * cbuf[rc]
            hc = o * np.tanh(c)
            cbuf[idx] = c
            hbuf[idx] = r * hc + (1.0 - r) * px[idx]
        h_fwd = hbuf[:n].copy()

        px = feats @ f["bw_Wp"][l].T + f["bw_bp"][l]
        x2 = feats @ f["bw_Wx"][l].T + f["bw_bx"][l]
        cbuf = np.zeros((n + 1, h), np.float32)
        hbuf = np.zeros((n + 1, h), np.float32)
        Wh, bh = f["bw_Wh"][l], f["bw_bh"][l]
        for idx in f["pre_order"]:
            p = f["parent"][idx]
            g = x2[idx] + hbuf[p] @ Wh.T + bh
            i_, o, fo, u, r = np.split(g, 5)
            i_, o, fo, r = sig(i_), sig(o), sig(fo), sig(r)
            u = np.tanh(u)
            c = i_ * u + fo * cbuf[p]
            hc = o * np.tanh(c)
            cbuf[idx] = c
            hbuf[idx] = r * hc + (1.0 - r) * px[idx]
        h_bwd = hbuf[:n].copy()
        feats = np.concatenate([h_fwd, h_bwd], axis=1)
    return feats


# ------------------------------------------------------------- bass program
def _build_nc():
    from contextlib import ExitStack

    import concourse.bacc as bacc
    import concourse.mybir as mybir
    import concourse.tile as tile

    F32 = mybir.dt.float32
    BF16 = mybir.dt.bfloat16
    F8 = mybir.dt.float8e4
    AF = mybir.ActivationFunctionType
    ALU = mybir.AluOpType
    SIG = AF.Sigmoid
    TANH = AF.Tanh

    nc = bacc.Bacc("TRN2", target_bir_lowering=False, debug=False,
                   num_devices=NCORES)

    featsT_d = nc.dram_tensor("featsT", [8, 128, NC], BF16, kind="ExternalInput")
    wl_d = nc.dram_tensor("wl", [L, 128, 4, 3072], BF16, kind="ExternalInput")
    wr_d = nc.dram_tensor("wr", [L, 128, 4, 3072], BF16, kind="ExternalInput")
    wh_d = nc.dram_tensor("wh", [L, 128, 4, 2560], F8, kind="ExternalInput")
    wxf_d = nc.dram_tensor("wxf", [L, 28, 128, 8, 128], BF16,
                           kind="ExternalInput")
    wxb_d = nc.dram_tensor("wxb", [L, 24, 128, 8, 128], BF16,
                           kind="ExternalInput")
    bf_d = nc.dram_tensor("bf", [L, 128, 28], F32, kind="ExternalInput")
    bb_d = nc.dram_tensor("bb", [L, 128, 24], F32, kind="ExternalInput")
    psel_d = nc.dram_tensor("psel", [128, 4], F32, kind="ExternalInput")
    lmask_d = nc.dram_tensor("lmask", [128, 1], F32, kind="ExternalInput")
    outT_d = nc.dram_tensor("outT", [8, 128, NC], F32, kind="ExternalOutput")
    rootc_d = nc.dram_tensor("rootc", [128, 4], F32, kind="ExternalOutput")
    f2top_d = nc.dram_tensor("f2top", [8, 128, 7], BF16, kind="ExternalOutput")

    with ExitStack() as ctx:
        tc = ctx.enter_context(tile.TileContext(nc))

        p_fb = ctx.enter_context(tc.tile_pool(name="fb", bufs=1))
        p_x2 = ctx.enter_context(tc.tile_pool(name="x2", bufs=1))
        p_ws = ctx.enter_context(tc.tile_pool(name="ws", bufs=1))
        p_wproj = ctx.enter_context(tc.tile_pool(name="wproj", bufs=3))
        p_bias = ctx.enter_context(tc.tile_pool(name="bias", bufs=2))
        p_gates = ctx.enter_context(tc.tile_pool(name="gates", bufs=2))
        p_tmp = ctx.enter_context(tc.tile_pool(name="tmp", bufs=2))
        p_cbuf = ctx.enter_context(tc.tile_pool(name="cbuf", bufs=2))
        p_small = ctx.enter_context(tc.tile_pool(name="small", bufs=2))
        p_hfs = ctx.enter_context(tc.tile_pool(name="hfs", bufs=1))
        p_psp = ctx.enter_context(tc.tile_pool(name="psp", bufs=2, space="PSUM"))
        p_pss = ctx.enter_context(tc.tile_pool(name="pss", bufs=5, space="PSUM"))
        p_psc = ctx.enter_context(tc.tile_pool(name="psc", bufs=1, space="PSUM"))
        p_dram = ctx.enter_context(tc.tile_pool(name="dram", bufs=2, space="DRAM"))

        # persistent feature/h storage (bf16): rows 128j..128j+127
        FB = []
        for j in range(8):
            t = p_fb.tile([128, NC], BF16, tag=f"fb{j}")
            nc.sync.dma_start(t[:], featsT_d[j])
            FB.append(t)
        psel_t = p_small.tile([128, 4], F32, tag="psel")
        nc.sync.dma_start(psel_t[:], psel_d[:])
        lmask_t = p_small.tile([128, 1], F32, tag="lmask")
        nc.sync.dma_start(lmask_t[:], lmask_d[:])

        CUR = {}   # current layer's scan weights
        ST = {}    # scan state: x2f/x2b tiles, hfs tiles, etc.

        # ---------------------------------------------------------- proj
        def alloc_layer(l):
            T = {}
            T["bft"] = p_bias.tile([128, 28], F32, tag="bf", name="bft")
            nc.sync.dma_start(T["bft"][:], bf_d[l])
            T["bbt"] = p_bias.tile([128, 24], F32, tag="bb", name="bbt")
            nc.sync.dma_start(T["bbt"][:], bb_d[l])
            T["x2f"] = p_x2.tile([128, 28, NC], BF16, tag="x2f", name="x2f")
            T["x2b"] = p_x2.tile([128, 24, NC], BF16, tag="x2b", name="x2b")
            return T

        def alloc_scan_weights(l):
            W = {}
            W["wh"] = p_ws.tile([128, 4, 2560], F8, tag="wh", name="wh")
            nc.sync.dma_start(W["wh"][:], wh_d[l])
            W["wl"] = p_ws.tile([128, 4, 3072], BF16, tag="wl", name="wl")
            nc.sync.dma_start(W["wl"][:], wl_d[l])
            W["wr"] = p_ws.tile([128, 4, 3072], BF16, tag="wr", name="wr")
            nc.sync.dma_start(W["wr"][:], wr_d[l])
            return W

        def proj_pass(l, T, direction, chunks, t0=0, t1=None):
            if direction == "b":
                nx, wx_d, bias_t, x2t = 24, wxb_d, T["bbt"], T["x2b"]
            else:
                nx, wx_d, bias_t, x2t = 28, wxf_d, T["bft"], T["x2f"]
            for t in range(t0, nx if t1 is None else t1):
                wt = p_wproj.tile([128, 8, 128], BF16, tag="wproj")
                nc.sync.dma_start(wt[:], wx_d[l, t])
                pss = [p_psp.tile([128, c1 - c0], F32, tag="psp",
                                   name=f"psp{ci}")
                       for ci, (c0, c1) in enumerate(chunks)]
                for k in range(8):
                    for ci, (c0, c1) in enumerate(chunks):
                        nc.tensor.matmul(pss[ci][:], wt[:, k, :],
                                         FB[k][:, c0:c1],
                                         start=(k == 0), stop=(k == 7))
                for ci, (c0, c1) in enumerate(chunks):
                    if ci % 2 == 0:
                        nc.scalar.activation(x2t[:, t, c0:c1], pss[ci][:],
                                             AF.Identity,
                                             bias=bias_t[:, t:t + 1])
                    else:
                        nc.vector.tensor_scalar(x2t[:, t, c0:c1], pss[ci][:],
                                                bias_t[:, t:t + 1], None,
                                                op0=ALU.add)

        # ---------------------------------------------------------- tails
        FUNCS_F = [SIG, SIG, SIG, SIG, SIG, TANH]   # i,o,fl,fr,r,u
        FUNCS_B = [SIG, SIG, SIG, SIG, TANH]        # i,o,f,r,u

        def out_write(m, off, hf, base, out_l, skip0):
            for j in range(4):
                eng = nc.gpsimd if (j % 2) else nc.vector
                eng.tensor_copy(FB[base + j][:, off:off + m], hf[:, j, :])
            if not out_l:
                return
            if off + m <= 134:
                for j in range(4):
                    eng = nc.vector if (j % 2) else nc.gpsimd
                    eng.tensor_copy(ST["hfs"][base + j][:, off:off + m],
                                    hf[:, j, :])
            else:
                s = 1 if skip0 else 0
                if m - s > 0:
                    for j in range(4):
                        nc.gpsimd.dma_start(
                            outT_d[base + j][:, off + s:off + m],
                            hf[:, j, s:m])

        def tail_fw(m, off, gsl, cl_ap, cr_ap, cnew_ap, out_l, skip0=False):
            gi, go, gfl, gfr, gr, gu = (gsl(q) for q in range(6))
            px = ST["x2f"][:, 24:28, off:off + m]
            nc.gpsimd.tensor_tensor(cnew_ap, gi, gu, op=ALU.mult)
            # off-critical: A = o*r, B = (1-r)*px
            A = p_tmp.tile([128, 4, m], F32, tag="A")
            nc.gpsimd.tensor_tensor(A[:], go, gr, op=ALU.mult)
            B = p_tmp.tile([128, 4, m], F32, tag="B")
            nc.gpsimd.tensor_tensor(B[:], gr, px, op=ALU.mult)
            nc.gpsimd.tensor_tensor(B[:], px, B[:], op=ALU.subtract)
            if cl_ap is not None:
                ct = p_tmp.tile([128, 4, m], F32, tag="ct")
                nc.vector.tensor_tensor(ct[:], gfl, cl_ap, op=ALU.mult)
                nc.vector.tensor_tensor(cnew_ap, cnew_ap, ct[:], op=ALU.add)
                if cr_ap is not None:
                    ct2 = p_tmp.tile([128, 4, m], F32, tag="ct")
                    nc.vector.tensor_tensor(ct2[:], gfr, cr_ap, op=ALU.mult)
                    nc.vector.tensor_tensor(cnew_ap, cnew_ap, ct2[:],
                                            op=ALU.add)
            th = p_tmp.tile([128, 4, m], F32, tag="th")
            nc.scalar.activation(th[:], cnew_ap, TANH)
            hf = p_tmp.tile([128, 4, m], F32, tag="hf")
            nc.vector.tensor_tensor(hf[:], A[:], th[:], op=ALU.mult)
            nc.vector.tensor_tensor(hf[:], hf[:], B[:], op=ALU.add)
            out_write(m, off, hf, 0, out_l, skip0)
            return hf

        def tail_bw(m, off, gsl, cp_ap, pairs, cnew_ap, out_l):
            gi, go, gf, gr, gu = (gsl(q) for q in range(5))
            px = ST["x2b"][:, 20:24, off:off + m]
            nc.gpsimd.tensor_tensor(cnew_ap, gi, gu, op=ALU.mult)
            A = p_tmp.tile([128, 4, m], F32, tag="A")
            nc.gpsimd.tensor_tensor(A[:], go, gr, op=ALU.mult)
            B = p_tmp.tile([128, 4, m], F32, tag="B")
            nc.gpsimd.tensor_tensor(B[:], gr, px, op=ALU.mult)
            nc.gpsimd.tensor_tensor(B[:], px, B[:], op=ALU.subtract)
            if cp_ap is not None:
                ct = p_tmp.tile([128, 4, m], F32, tag="ct")
                if pairs:
                    nc.vector.tensor_tensor(ct[:, :, 0:m:2],
                                            gf[:, :, 0:m:2], cp_ap,
                                            op=ALU.mult)
                    nc.vector.tensor_tensor(ct[:, :, 1:m:2],
                                            gf[:, :, 1:m:2], cp_ap,
                                            op=ALU.mult)
                else:
                    nc.vector.tensor_tensor(ct[:], gf, cp_ap, op=ALU.mult)
                nc.vector.tensor_tensor(cnew_ap, cnew_ap, ct[:], op=ALU.add)
            th = p_tmp.tile([128, 4, m], F32, tag="th")
            nc.scalar.activation(th[:], cnew_ap, TANH)
            hf = p_tmp.tile([128, 4, m], F32, tag="hf")
            nc.vector.tensor_tensor(hf[:], A[:], th[:], op=ALU.mult)
            nc.vector.tensor_tensor(hf[:], hf[:], B[:], op=ALU.add)
            out_write(m, off, hf, 4, out_l, False)
            return hf

        # ------------------------------------------------------- fw steps
        def leaf_fw(m, off, cnew_ap, out_l, skip0=False):
            x2f = ST["x2f"]
            if m <= 16:
                A = p_gates.tile([128, 24, m], F32, tag="g0")
                nc.scalar.activation(A[:, 0:20, :],
                                     x2f[:, 0:20, off:off + m], SIG)
                nc.scalar.activation(A[:, 20:24, :],
                                     x2f[:, 20:24, off:off + m], TANH)
                gsl = lambda q: A[:, 4 * q:4 * q + 4, :]
            else:
                acts = []
                for q in range(6):
                    a = p_gates.tile([128, 4, m], F32, tag=f"g{q}")
                    nc.scalar.activation(a[:], x2f[:, 4 * q:4 * q + 4,
                                                   off:off + m], FUNCS_F[q])
                    acts.append(a)
                gsl = lambda q: acts[q][:]
            return tail_fw(m, off, gsl, None, None, cnew_ap, out_l, skip0)

        def step_fw(m, off, hl_fn, hr_fn, cl_ap, cr_ap, cnew_ap, out_l,
                    use_wr=True, skip0=False):
            """hl_fn/hr_fn: k -> [128, m] rhs AP (hr_fn None with use_wr False)."""
            x2f = ST["x2f"]
            wl, wr = CUR["wl"], CUR["wr"]
            nmm = 8 if use_wr else 4
            if m <= 16:
                ps = p_psc.tile([128, 24, m], F32, tag="psc")
                for t in range(24):
                    for k in range(4):
                        nc.tensor.matmul(ps[:, t, :],
                                         wl[:, k, 128 * t:128 * (t + 1)],
                                         hl_fn(k), start=(k == 0),
                                         stop=(k == nmm - 1))
                    if use_wr:
                        for k in range(4):
                            nc.tensor.matmul(ps[:, t, :],
                                             wr[:, k, 128 * t:128 * (t + 1)],
                                             hr_fn(k), start=False,
                                             stop=(k == 3))
                g = p_tmp.tile([128, 24, m], F32, tag="g")
                nc.vector.tensor_tensor(g[:], ps[:],
                                        x2f[:, 0:24, off:off + m], op=ALU.add)
                A = p_gates.tile([128, 24, m], F32, tag="g0")
                nc.scalar.activation(A[:, 0:20, :], g[:, 0:20, :], SIG)
                nc.scalar.activation(A[:, 20:24, :], g[:, 20:24, :], TANH)
                gsl = lambda q: A[:, 4 * q:4 * q + 4, :]
            else:
                acts = []
                for q in range(6):
                    ps = p_pss.tile([128, 4, m], F32, tag="pss")
                    for j4 in range(4):
                        t = 4 * q + j4
                        for k in range(4):
                            nc.tensor.matmul(ps[:, j4, :],
                                             wl[:, k, 128 * t:128 * (t + 1)],
                                             hl_fn(k), start=(k == 0),
                                             stop=(k == nmm - 1))
                        if use_wr:
                            for k in range(4):
                                nc.tensor.matmul(
                                    ps[:, j4, :],
                                    wr[:, k, 128 * t:128 * (t + 1)],
                                    hr_fn(k), start=False, stop=(k == 3))
                    g = p_tmp.tile([128, 4, m], F32, tag="g")
                    nc.vector.tensor_tensor(
                        g[:], ps[:], x2f[:, 4 * q:4 * q + 4, off:off + m],
                        op=ALU.add)
                    a = p_gates.tile([128, 4, m], F32, tag=f"g{q}")
                    nc.scalar.activation(a[:], g[:], FUNCS_F[q])
                    acts.append(a)
                gsl = lambda q: acts[q][:]
            return tail_fw(m, off, gsl, cl_ap, cr_ap, cnew_ap, out_l, skip0)

        def chunks_of(m):
            # keep whole levels in one call; only bound tile sizes at 128
            return [m] if m <= 128 else [128] * (m // 128)

        def step_fw_lvl(lvl, cc_child, out_l):
            """Generic fw level step reading children from FB / child c tile."""
            m = 1 << (lvl - 3)
            off = _lvl_off(lvl)
            offc = _lvl_off(lvl + 1)
            cn = p_cbuf.tile([128, 4, m], F32, tag="cfw")
            hf = None
            c0 = 0
            for mc in chunks_of(m):
                o2 = offc + 2 * c0
                hf = step_fw(
                    mc, off + c0,
                    lambda k, o2=o2, mc=mc: FB[k][:, o2:o2 + 2 * mc:2],
                    lambda k, o2=o2, mc=mc: FB[k][:, o2 + 1:o2 + 2 * mc:2],
                    cc_child[:, :, 2 * c0:2 * (c0 + mc):2],
                    cc_child[:, :, 2 * c0 + 1:2 * (c0 + mc):2],
                    cn[:, :, c0:c0 + mc], out_l)
                c0 += mc
            return hf, cn

        # ------------------------------------------------------- bw steps
        def leaf_bw0(out_l):
            """node 0: no parent (zero slot) -> gates from x2 only."""
            x2b = ST["x2b"]
            A = p_gates.tile([128, 20, 1], F32, tag="g0")
            nc.scalar.activation(A[:, 0:16, :], x2b[:, 0:16, 0:1], SIG)
            nc.scalar.activation(A[:, 16:20, :], x2b[:, 16:20, 0:1], TANH)
            gsl = lambda q: A[:, 4 * q:4 * q + 4, :]
            cn = p_cbuf.tile([128, 4, 1], F32, tag="cbw")
            hf = tail_bw(1, 0, gsl, None, False, cn[:], out_l)
            return hf, cn

        def step_bw(m, off, hp_fn, mp, cp_ap, cnew_ap, out_l):
            """m kids at cols off.., mp parents; hp_fn: k -> [128, mp] AP."""
            x2b = ST["x2b"]
            wh = CUR["wh"]
            pairs = m > mp
            if mp <= 16:
                ps = p_psc.tile([128, 20, mp], F32, tag="psc")
                for t in range(20):
                    for k in range(4):
                        nc.tensor.matmul(ps[:, t, :],
                                         wh[:, k, 128 * t:128 * (t + 1)],
                                         hp_fn(k), start=(k == 0),
                                         stop=(k == 3))
                g = p_tmp.tile([128, 20, m], F32, tag="g")
                if pairs:
                    nc.vector.tensor_tensor(
                        g[:, :, 0:m:2], ps[:],
                        x2b[:, 0:20, off:off + m:2], op=ALU.add)
                    nc.vector.tensor_tensor(
                        g[:, :, 1:m:2], ps[:],
                        x2b[:, 0:20, off + 1:off + m:2], op=ALU.add)
                else:
                    nc.vector.tensor_tensor(g[:], ps[:],
                                            x2b[:, 0:20, off:off + m],
                                            op=ALU.add)
                A = p_gates.tile([128, 20, m], F32, tag="g0")
                nc.scalar.activation(A[:, 0:16, :], g[:, 0:16, :], SIG)
                nc.scalar.activation(A[:, 16:20, :], g[:, 16:20, :], TANH)
                gsl = lambda q: A[:, 4 * q:4 * q + 4, :]
            else:
                acts = []
                for q in range(5):
                    ps = p_pss.tile([128, 4, mp], F32, tag="pss")
                    for j4 in range(4):
                        t = 4 * q + j4
                        for k in range(4):
                            nc.tensor.matmul(ps[:, j4, :],
                                             wh[:, k, 128 * t:128 * (t + 1)],
                                             hp_fn(k), start=(k == 0),
                                             stop=(k == 3))
                    g = p_tmp.tile([128, 4, m], F32, tag="g")
                    if pairs:
                        nc.vector.tensor_tensor(
                            g[:, :, 0:m:2], ps[:],
                            x2b[:, 4 * q:4 * q + 4, off:off + m:2],
                            op=ALU.add)
                        nc.vector.tensor_tensor(
                            g[:, :, 1:m:2], ps[:],
                            x2b[:, 4 * q:4 * q + 4, off + 1:off + m:2],
                            op=ALU.add)
                    else:
                        nc.vector.tensor_tensor(
                            g[:], ps[:],
                            x2b[:, 4 * q:4 * q + 4, off:off + m], op=ALU.add)
                    a = p_gates.tile([128, 4, m], F32, tag=f"g{q}")
                    nc.scalar.activation(a[:], g[:], FUNCS_B[q])
                    acts.append(a)
                gsl = lambda q: acts[q][:]
            return tail_bw(m, off, gsl, cp_ap, pairs, cnew_ap, out_l)

        def step_bw_lvl(lvl, cprev, out_l):
            """Generic bw level step (lvl >= 4): parents at lvl-1 cols."""
            m = 1 << (lvl - 3)
            off = _lvl_off(lvl)
            offp = _lvl_off(lvl - 1)
            cn = p_cbuf.tile([128, 4, m], F32, tag="cbw")
            hf = None
            c0 = 0
            for mc in chunks_of(m):
                mpc = mc // 2
                hf = step_bw(
                    mc, off + c0,
                    lambda k, p0=offp + c0 // 2, mpc=mpc:
                        FB[4 + k][:, p0:p0 + mpc],
                    mpc, cprev[:, :, c0 // 2:c0 // 2 + mpc],
                    cn[:, :, c0:c0 + mc], out_l)
                c0 += mc
            return hf, cn

        # ------------------------------------------------------- scans
        def scans_emit(l, out_l):
            x2f, x2b = ST["x2f"], ST["x2b"]

            # fw lvl12 leaf (node 4095 slot, col 518)
            c12 = p_small.tile([128, 4, 1], F32, tag="c12")
            h12 = leaf_fw(1, 518, c12[:], out_l)

            # bw top: nodes 0..6
            h_b0, cb0 = leaf_bw0(out_l)
            cb1 = p_cbuf.tile([128, 4, 2], F32, tag="cbw")
            h_b1 = step_bw(2, 1, lambda k: FB[4 + k][:, 0:1], 1,
                           cb0[:, :, 0:1], cb1[:], out_l)
            cb2 = p_cbuf.tile([128, 4, 4], F32, tag="cbw")
            h_b2 = step_bw(4, 3, lambda k: FB[4 + k][:, 1:3], 2,
                           cb1[:, :, 0:2], cb2[:], out_l)

            # bw lvl3 (subtree root, col 7): psel one-hot parent selection.
            # Emitted before the leaf floods so its DVE chain isn't queued
            # behind the leaf tails.
            hps = p_small.tile([128, 4, 1], BF16, tag="hps")
            cps = p_small.tile([128, 4, 1], F32, tag="cps")
            hsel = p_small.tile([128, 4, 1], F32, tag="hsel")
            for j in range(4):
                tsel = p_small.tile([128, 4], F32, tag="tsel")
                nc.vector.tensor_tensor(tsel[:], h_b2[:, j, :], psel_t[:],
                                        op=ALU.mult)
                nc.vector.tensor_reduce(hsel[:, j, :], tsel[:],
                                        mybir.AxisListType.X, ALU.add)
                tsel2 = p_small.tile([128, 4], F32, tag="tsel2")
                nc.vector.tensor_tensor(tsel2[:], cb2[:, j, :], psel_t[:],
                                        op=ALU.mult)
                nc.vector.tensor_reduce(cps[:, j, :], tsel2[:],
                                        mybir.AxisListType.X, ALU.add)
            nc.vector.tensor_copy(hps[:], hsel[:])
            cb3 = p_cbuf.tile([128, 4, 1], F32, tag="cbw")
            step_bw(1, 7, lambda k: hps[:, k, :], 1, cps[:], cb3[:], out_l)

            # fw lvl11 leaf chunk 1 (cols 262..389) + col-262 correction
            c11 = p_cbuf.tile([128, 4, 256], F32, tag="cfw")
            leaf_fw(128, 262, c11[:, :, 0:128], out_l, skip0=True)
            hlc = p_small.tile([128, 4, 1], BF16, tag="hlc")
            clc = p_small.tile([128, 4, 1], F32, tag="clc")
            nc.vector.tensor_scalar(hlc[:], h12[:], lmask_t[:], None,
                                    op0=ALU.mult)
            nc.vector.tensor_scalar(clc[:], c12[:], lmask_t[:], None,
                                    op0=ALU.mult)
            step_fw(1, 262, lambda k: hlc[:, k, :], None, clc[:], None,
                    c11[:, :, 0:1], out_l, use_wr=False)

            # fw lvl11 leaf chunk 2
            leaf_fw(128, 390, c11[:, :, 128:256], out_l)

            st = {"cf": c11, "cb": cb3, "hro": None, "cro": None}

            def emit_bw(lvl):
                if lvl == 12:
                    cn = p_small.tile([128, 4, 1], F32, tag="c12b")
                    step_bw(1, 518, lambda k: FB[4 + k][:, 262:263], 1,
                            st["cb"][:, :, 0:1], cn[:], out_l)
                    return
                hf, cn = step_bw_lvl(lvl, st["cb"], out_l)
                st["cb"] = cn

            def emit_fw(lvl):
                hf, cn = step_fw_lvl(lvl, st["cf"], out_l)
                st["hro"], st["cf"] = hf, cn

            for blvl, flvl in ((4, 10), (5, 9), (6, 8), (7, 7), (8, 6),
                               (9, 5), (10, 4), (11, 3)):
                emit_bw(blvl)
                emit_fw(flvl)
            emit_bw(12)
            return st["hro"], st["cf"]

        def fwtop_emit(hroots_bf, croots, out_l):
            ct2 = p_small.tile([128, 4, 4], F32, tag="ct2")
            h2 = step_fw(4, 3,
                         lambda k: hroots_bf[:, k, 0:8:2],
                         lambda k: hroots_bf[:, k, 1:8:2],
                         croots[:, :, 0:8:2], croots[:, :, 1:8:2],
                         ct2[:], out_l)
            ct1 = p_small.tile([128, 4, 2], F32, tag="ct1")
            h1 = step_fw(2, 1,
                         lambda k: FB[k][:, 3:7:2],
                         lambda k: FB[k][:, 4:7:2],
                         ct2[:, :, 0:4:2], ct2[:, :, 1:4:2],
                         ct1[:], out_l)
            ct0 = p_small.tile([128, 4, 1], F32, tag="ct0")
            step_fw(1, 0,
                    lambda k: FB[k][:, 1:3:2],
                    lambda k: FB[k][:, 2:3:2],
                    ct1[:, :, 0:1], ct1[:, :, 1:2],
                    ct0[:], out_l)

        # =================================================== layer 0
        T0 = alloc_layer(0)
        ST.update(x2f=T0["x2f"], x2b=T0["x2b"])
        W0 = alloc_scan_weights(0)
        CUR.update(W0)
        proj_pass(0, T0, "b", [(0, 260), (260, 520)])
        proj_pass(0, T0, "f", [(0, 260), (260, 520)])
        # snapshot the fw head (cols 0..6) so fwtop doesn't pin the x2f buffer
        hd0 = p_small.tile([128, 28, 7], BF16, tag="hd0")
        nc.vector.tensor_copy(hd0[:], T0["x2f"][:, :, 0:7])
        hro0, cf0 = scans_emit(0, False)

        # allgather subtree roots of layer 0
        ccin = p_dram.tile([1024], F32, tag="ccin")
        ccout = p_dram.tile([NCORES, 1024], F32, tag="ccout")
        ccin_v = ccin[:].rearrange("(j p) -> p j", p=128)
        nc.gpsimd.dma_start(ccin_v[:, 0:4], hro0[:, :, 0])
        nc.gpsimd.dma_start(ccin_v[:, 4:8], cf0[:, :, 0])
        nc.gpsimd.collective_compute(
            "AllGather", mybir.AluOpType.bypass,
            replica_groups=[list(range(NCORES))],
            ins=[ccin[:].opt()], outs=[ccout[:].opt()])
        ccout_v = ccout[:].rearrange("r (j p) -> p j r", p=128)
        hroots = p_small.tile([128, 4, 8], F32, tag="hroots")
        croots = p_small.tile([128, 4, 8], F32, tag="croots")
        for j in range(4):
            nc.gpsimd.dma_start(hroots[:, j, :], ccout_v[:, j, :])
            nc.gpsimd.dma_start(croots[:, j, :], ccout_v[:, 4 + j, :])
        hroots_bf = p_small.tile([128, 4, 8], BF16, tag="hrootsb")
        nc.vector.tensor_copy(hroots_bf[:], hroots[:])

        # =================================================== layer 1
        T1 = alloc_layer(1)
        ST.update(x2f=T1["x2f"], x2b=T1["x2b"])
        hfs = [p_hfs.tile([128, 134], F32, tag=f"hfs{j}", name=f"hfs{j}")
               for j in range(8)]
        ST["hfs"] = hfs

        # main passes (subtree cols; overlap the collective)
        proj_pass(1, T1, "b", [(7, 264), (264, 520)])
        proj_pass(1, T1, "f", [(7, 264), (264, 520)])

        # finish layer 0: redundant top-7 fw scan (layer-0 weights, head x2)
        ST.update(x2f=hd0)
        fwtop_emit(hroots_bf, croots, False)
        ST.update(x2f=T1["x2f"])

        # bw head pass (cols 0..6 need fwtop-0)
        proj_pass(1, T1, "b", [(0, 7)])
        for j in range(8):
            nc.sync.dma_start(f2top_d[j], FB[j][:, 0:7])

        W1 = alloc_scan_weights(1)
        CUR.update(W1)
        hro1, cf1 = scans_emit(1, True)
        nc.sync.dma_start(rootc_d[:], cf1[:, :, 0])
        for j in range(8):
            nc.gpsimd.dma_start(outT_d[j][:, 0:134], hfs[j][:])

    nc.compile()
    return nc


# ------------------------------------------------------------------ packing
def _pack_inputs(inp):
    def bf(x):
        return np.ascontiguousarray(x).astype(BF16NP)

    feats = np.asarray(inp["features"], np.float32)
    per_core = []
    wl = np.stack([np.asarray(inp["fw_Wl"][l], np.float32)[_PF].T
                   .reshape(4, 128, 3072).transpose(1, 0, 2) for l in range(L)])
    wr = np.stack([np.asarray(inp["fw_Wr"][l], np.float32)[_PF].T
                   .reshape(4, 128, 3072).transpose(1, 0, 2) for l in range(L)])
    wh = np.stack([np.asarray(inp["bw_Wh"][l], np.float32)[_PB].T
                   .reshape(4, 128, 2560).transpose(1, 0, 2) for l in range(L)])

    def proj_pack(w):
        # w = W.T [1024, M] -> [M/128, 128p, 8k, 128m]
        M = w.shape[1]
        v = w.reshape(8, 128, M // 128, 128)  # (k, p, t, m)
        return np.ascontiguousarray(v.transpose(2, 1, 0, 3))

    wxf = np.stack([
        proj_pack(np.concatenate(
            [np.asarray(inp["fw_Wx"][l], np.float32)[_PF],
             np.asarray(inp["fw_Wp"][l], np.float32)], 0).T)
        for l in range(L)])
    wxb = np.stack([
        proj_pack(np.concatenate(
            [np.asarray(inp["bw_Wx"][l], np.float32)[_PB],
             np.asarray(inp["bw_Wp"][l], np.float32)], 0).T)
        for l in range(L)])
    bfv = np.stack([
        np.concatenate([
            (np.asarray(inp["fw_bx"][l], np.float32)
             + np.asarray(inp["fw_bl"][l], np.float32)
             + np.asarray(inp["fw_br"][l], np.float32))[_PF],
            np.asarray(inp["fw_bp"][l], np.float32)], 0)
        .reshape(28, 128).T for l in range(L)])
    bbv = np.stack([
        np.concatenate([
            (np.asarray(inp["bw_bx"][l], np.float32)
             + np.asarray(inp["bw_bh"][l], np.float32))[_PB],
            np.asarray(inp["bw_bp"][l], np.float32)], 0)
        .reshape(24, 128).T for l in range(L)])
    base = {
        "wl": bf(wl), "wr": bf(wr),
        "wh": np.ascontiguousarray(wh).astype(F8NP),
        "wxf": bf(wxf), "wxb": bf(wxb),
        "bf": np.ascontiguousarray(bfv, dtype=np.float32),
        "bb": np.ascontiguousarray(bbv, dtype=np.float32),
    }
    for c in range(NCORES):
        cm = _col_map(c)
        v = cm >= 0
        fT = np.zeros((1024, NC), np.float32)
        fT[:, v] = feats[cm[v]].T
        psel = np.zeros((128, 4), np.float32)
        psel[:, c // 2] = 1.0
        lmask = np.full((128, 1), 1.0 if c == 0 else 0.0, np.float32)
        m = dict(base)
        m["featsT"] = bf(fT.reshape(8, 128, NC))
        m["psel"] = psel
        m["lmask"] = lmask
        per_core.append(m)
    return per_core


def _host_fwtop(inp, results):
    """Compute the final layer's top-7 forward h on the host, mirroring the
    device arithmetic (bf16 matmul inputs, fp32 accumulation)."""
    l = L - 1

    def bf(x):
        return x.astype(BF16NP).astype(np.float32)

    def sig(x):
        return 1.0 / (1.0 + np.exp(-x))

    # features of layer 1 at nodes 0..6 (bf16 as on device)
    f2 = np.concatenate([np.asarray(results[0]["f2top"], np.float32)[j]
                         for j in range(8)], 0)  # [1024, 7]
    wxf = np.concatenate([np.asarray(inp["fw_Wx"][l], np.float32),
                          np.asarray(inp["fw_Wp"][l], np.float32)], 0)
    bxf = np.concatenate([
        np.asarray(inp["fw_bx"][l], np.float32)
        + np.asarray(inp["fw_bl"][l], np.float32)
        + np.asarray(inp["fw_br"][l], np.float32),
        np.asarray(inp["fw_bp"][l], np.float32)], 0)
    pf = bf(wxf) @ f2 + bxf[:, None]  # f2 already bf16-rounded
    x2, px = pf[:3072], pf[3072:]
    wl = bf(np.asarray(inp["fw_Wl"][l], np.float32))
    wr = bf(np.asarray(inp["fw_Wr"][l], np.float32))

    # subtree roots: h from outT col 7 (f32), c from rootc
    hr8 = np.stack([np.asarray(results[c]["outT"], np.float32)
                    .reshape(1024, NC)[0:512, 7] for c in range(NCORES)], 1)
    cr8 = np.stack([np.asarray(results[c]["rootc"], np.float32)
                    .T.reshape(512) for c in range(NCORES)], 1)

    hbuf = np.zeros((512, 7), np.float32)
    cbuf = np.zeros((512, 7), np.float32)

    def step(cols, hl, hr, cl, cr):
        g = x2[:, cols] + wl @ bf(hl) + wr @ bf(hr)
        i_, o, fl, fr, u, r = (g[k * 512:(k + 1) * 512] for k in range(6))
        i_, o, fl, fr, r = sig(i_), sig(o), sig(fl), sig(fr), sig(r)
        u = np.tanh(u)
        cc = i_ * u + fl * cl + fr * cr
        hc = o * np.tanh(cc)
        return cc, hc * r + (1.0 - r) * px[:, cols]

    cc, hf = step([3, 4, 5, 6], hr8[:, 0::2], hr8[:, 1::2],
                  cr8[:, 0::2], cr8[:, 1::2])
    hbuf[:, 3:7], cbuf[:, 3:7] = hf, cc
    cc, hf = step([1, 2], hbuf[:, 3:7:2], hbuf[:, 4:7:2],
                  cbuf[:, 3:7:2], cbuf[:, 4:7:2])
    hbuf[:, 1:3], cbuf[:, 1:3] = hf, cc
    cc, hf = step([0], hbuf[:, 1:2], hbuf[:, 2:3],
                  cbuf[:, 1:2], cbuf[:, 2:3])
    hbuf[:, 0:1] = hf
    return hbuf  # [512, 7]


def _assemble(inp, results):
    out = np.zeros((N, 1024), np.float32)
    for c in range(NCORES):
        cm = _col_map(c)
        o = np.asarray(results[c]["outT"]).reshape(1024, NC)
        cols = np.arange(NC)
        use = (cm >= 0) & (cols >= 7) & (cols != 519)
        if c != 0:
            use &= cols != 518
        out[cm[use]] = o[:, use].T
        if c == 0:
            out[0:7, 512:1024] = o[512:1024, 0:7].T  # bw half from device
    out[0:7, 0:512] = _host_fwtop(inp, results).T
    return out


def kernel(**inputs):
    inp = {k: np.asarray(v) for k, v in inputs.items()}
    if not _is_canonical(inp):
        return _fallback(inp)
    if "nc" not in _CACHE:
        _CACHE["nc"] = _build_nc()
    from concourse.bass_utils import run_bass_kernel_spmd

    in_maps = _pack_inputs(inp)
    res = run_bass_kernel_spmd(_CACHE["nc"], in_maps,
                               core_ids=list(range(NCORES)))
    return _assemble(inp, res.results)


if __name__ == "__main__":
    d = np.load("/tmp/inputs.npz")
    inputs = {k: d[k] for k in d.files}
    expected = np.load("/tmp/expected.npy")
    actual = kernel(**inputs)
    err = np.abs(actual - expected)
    print("max abs err:", err.max())
    print("absmax-rel:", err.max() / np.abs(expected).max())
    print("mean abs:", err.mean())


# revision 3
# speedup vs baseline: 1.1584x; 1.1584x over previous
"""Trainium2 Bass kernel for MultiLayer bidirectional BTreeLSTM (4096-node
balanced heap tree, IN=OUT=1024, H=512, L=2).

v2 strategy (vs v1 baseline):
- x2/px projections stay SBUF-resident in bf16 (no DRAM round-trip), one
  weight pass per layer with two 260-col matmul chunks per LDWEIGHTS.
- Scan matmuls read h directly from the persistent FB tiles via strided
  APs (no staging copies); the backward direction multiplies over the
  m/2 parent columns only (both children share W@h_parent) and the
  per-child x2 add re-expands via strided adds.
- Gate blocks are reordered host-side to [i,o,fl,fr,r,u] (fw) and
  [i,o,f,r,u] (bw) so small levels (m<=16) run as one PSUM tile with a
  single gate-sum add and just two activation instructions.
- Small-level outputs accumulate in an f32 SBUF strip (cols 0..133) and
  ship to DRAM in one DMA per 128-row block.

Per-core column layout (NC=520): [0..6]=nodes 0..6, [7..517]=subtree
levels 3..11 in level order, [518]=level-12 slot (node 4095, core 0 only),
[519]=pad.
"""

import numpy as np
import ml_dtypes

F8NP = ml_dtypes.float8_e4m3fn
N = 4096
H = 512
L = 2
NCORES = 8
NC = 520
BF16NP = ml_dtypes.bfloat16

_CACHE = {}

# gate-block permutations (rows of the 6H/5H gate dim)
_PF = np.concatenate([np.arange(512 * b, 512 * (b + 1))
                      for b in (0, 1, 2, 3, 5, 4)])  # i,o,fl,fr,r,u
_PB = np.concatenate([np.arange(512 * b, 512 * (b + 1))
                      for b in (0, 1, 2, 4, 3)])     # i,o,f,r,u


# ----------------------------------------------------------------- host utils
def _lvl_off(lvl):
    return 7 + (1 << (lvl - 3)) - 1


def _col_map(core):
    r = 7 + core
    ids = list(range(7))
    for lvl in range(3, 12):
        w = 1 << (lvl - 3)
        start = (r + 1) * w - 1
        ids.extend(range(start, start + w))
    ids.append(4095 if core == 0 else -1)
    ids.append(-1)
    return np.array(ids, np.int64)


def _is_canonical(inp):
    n = N
    i = np.arange(n)
    left = np.where(2 * i + 1 < n, 2 * i + 1, n).astype(np.int32)
    right = np.where(2 * i + 2 < n, 2 * i + 2, n).astype(np.int32)
    parent = np.where(i > 0, (i - 1) // 2, n).astype(np.int32)
    if inp["features"].shape != (N, 1024):
        return False
    for k, v in (("left_child", left), ("right_child", right), ("parent", parent)):
        if inp[k].shape != (n,) or not np.array_equal(np.asarray(inp[k]), v):
            return False
    po = np.asarray(inp["post_order"])
    pr = np.asarray(inp["pre_order"])
    if sorted(po.tolist()) != list(range(n)) or sorted(pr.tolist()) != list(range(n)):
        return False
    pos = np.empty(n, np.int64)
    pos[po] = np.arange(n)
    ok = True
    for child in (left, right):
        m = child < n
        ok &= bool((pos[i[m]] > pos[child[m]]).all())
    pos[pr] = np.arange(n)
    m = parent < n
    ok &= bool((pos[i[m]] > pos[parent[m]]).all())
    return ok


def _fallback(inp):
    """Literal numpy re-implementation of the reference scan (any inputs)."""
    f = {k: np.asarray(v) for k, v in inp.items()}
    feats = f["features"].astype(np.float32)
    n = feats.shape[0]

    def sig(x):
        return 1.0 / (1.0 + np.exp(-x))

    for l in range(L):
        h = f["fw_bp"][l].shape[0]
        px = feats @ f["fw_Wp"][l].T + f["fw_bp"][l]
        x2 = feats @ f["fw_Wx"][l].T + f["fw_bx"][l]
        cbuf = np.zeros((n + 1, h), np.float32)
        hbuf = np.zeros((n + 1, h), np.float32)
        Wl, bl, Wr, br = f["fw_Wl"][l], f["fw_bl"][l], f["fw_Wr"][l], f["fw_br"][l]
        for idx in f["post_order"]:
            lc, rc = f["left_child"][idx], f["right_child"][idx]
            g = x2[idx] + hbuf[lc] @ Wl.T + bl + hbuf[rc] @ Wr.T + br
            i_, o, fl, fr, u, r = np.split(g, 6)
            i_, o, fl, fr, r = sig(i_), sig(o), sig(fl), sig(fr), sig(r)
            u = np.tanh(u)
            c = i_ * u + fl * cbuf[lc] + fr * cbuf[rc]
            hc = o * np.tanh(c)
            cbuf[idx] = c
            hbuf[idx] = r * hc + (1.0 - r) * px[idx]
        h_fwd = hbuf[:n].copy()

        px = feats @ f["bw_Wp"][l].T + f["bw_bp"][l]
        x2 = feats @ f["bw_Wx"][l].T + f["bw_bx"][l]
        cbuf = np.zeros((n + 1, h), np.float32)
        hbuf = np.zeros((n + 1, h), np.float32)
        Wh, bh = f["bw_Wh"][l], f["bw_bh"][l]
        for idx in f["pre_order"]:
            p = f["parent"][idx]
            g = x2[idx] + hbuf[p] @ Wh.T + bh
            i_, o, fo, u, r = np.split(g, 5)
            i_, o, fo, r = sig(i_), sig(o), sig(fo), sig(r)
            u = np.tanh(u)
            c = i_ * u + fo * cbuf[p]
            hc = o * np.tanh(c)
            cbuf[idx] = c
            hbuf[idx] = r * hc + (1.0 - r) * px[idx]
        h_bwd = hbuf[:n].copy()
        feats = np.concatenate([h_fwd, h_bwd], axis=1)
    return feats


# ------------------------------------------------------------- bass program
def _build_nc():
    from contextlib import ExitStack

    import concourse.bacc as bacc
    import concourse.mybir as mybir
    import concourse.tile as tile

    F32 = mybir.dt.float32
    BF16 = mybir.dt.bfloat16
    F8 = mybir.dt.float8e4
    AF = mybir.ActivationFunctionType
    ALU = mybir.AluOpType
    SIG = AF.Sigmoid
    TANH = AF.Tanh

    nc = bacc.Bacc("TRN2", target_bir_lowering=False, debug=False,
                   num_devices=NCORES)

    featsT_d = nc.dram_tensor("featsT", [8, 128, NC], BF16, kind="ExternalInput")
    wl_d = nc.dram_tensor("wl", [L, 128, 4, 3072], BF16, kind="ExternalInput")
    wr_d = nc.dram_tensor("wr", [L, 128, 4, 3072], BF16, kind="ExternalInput")
    wh_d = nc.dram_tensor("wh", [L, 128, 4, 2560], BF16, kind="ExternalInput")
    wxf_d = nc.dram_tensor("wxf", [L, 28, 128, 8, 128], BF16,
                           kind="ExternalInput")
    wxb_d = nc.dram_tensor("wxb", [L, 24, 128, 8, 128], BF16,
                           kind="ExternalInput")
    bf_d = nc.dram_tensor("bf", [L, 128, 28], F32, kind="ExternalInput")
    bb_d = nc.dram_tensor("bb", [L, 128, 24], F32, kind="ExternalInput")
    psel_d = nc.dram_tensor("psel", [128, 4], F32, kind="ExternalInput")
    lmask_d = nc.dram_tensor("lmask", [128, 1], F32, kind="ExternalInput")
    outT_d = nc.dram_tensor("outT", [8, 128, NC], F32, kind="ExternalOutput")
    rootc_d = nc.dram_tensor("rootc", [128, 4], F32, kind="ExternalOutput")
    f2top_d = nc.dram_tensor("f2top", [8, 128, 7], BF16, kind="ExternalOutput")

    with ExitStack() as ctx:
        tc = ctx.enter_context(tile.TileContext(nc))

        p_fb = ctx.enter_context(tc.tile_pool(name="fb", bufs=1))
        p_x2 = ctx.enter_context(tc.tile_pool(name="x2", bufs=1))
        p_ws = ctx.enter_context(tc.tile_pool(name="ws", bufs=1))
        p_wproj = ctx.enter_context(tc.tile_pool(name="wproj", bufs=3))
        p_bias = ctx.enter_context(tc.tile_pool(name="bias", bufs=2))
        p_gates = ctx.enter_context(tc.tile_pool(name="gates", bufs=2))
        p_tmp = ctx.enter_context(tc.tile_pool(name="tmp", bufs=2))
        p_cbuf = ctx.enter_context(tc.tile_pool(name="cbuf", bufs=2))
        p_small = ctx.enter_context(tc.tile_pool(name="small", bufs=2))
        p_hfs = ctx.enter_context(tc.tile_pool(name="hfs", bufs=1))
        p_psp = ctx.enter_context(tc.tile_pool(name="psp", bufs=2, space="PSUM"))
        p_pss = ctx.enter_context(tc.tile_pool(name="pss", bufs=5, space="PSUM"))
        p_psc = ctx.enter_context(tc.tile_pool(name="psc", bufs=1, space="PSUM"))
        p_dram = ctx.enter_context(tc.tile_pool(name="dram", bufs=2, space="DRAM"))

        # persistent feature/h storage (bf16): rows 128j..128j+127
        FB = []
        for j in range(8):
            t = p_fb.tile([128, NC], BF16, tag=f"fb{j}")
            nc.sync.dma_start(t[:], featsT_d[j])
            FB.append(t)
        psel_t = p_small.tile([128, 4], F32, tag="psel")
        nc.sync.dma_start(psel_t[:], psel_d[:])
        lmask_t = p_small.tile([128, 1], F32, tag="lmask")
        nc.sync.dma_start(lmask_t[:], lmask_d[:])

        CUR = {}   # current layer's scan weights
        ST = {}    # scan state: x2f/x2b tiles, hfs tiles, etc.

        # ---------------------------------------------------------- proj
        def alloc_layer(l):
            T = {}
            T["bft"] = p_bias.tile([128, 28], F32, tag="bf", name="bft")
            nc.sync.dma_start(T["bft"][:], bf_d[l])
            T["bbt"] = p_bias.tile([128, 24], F32, tag="bb", name="bbt")
            nc.sync.dma_start(T["bbt"][:], bb_d[l])
            T["x2f"] = p_x2.tile([128, 28, NC], BF16, tag="x2f", name="x2f")
            T["x2b"] = p_x2.tile([128, 24, NC], BF16, tag="x2b", name="x2b")
            return T

        def alloc_scan_weights(l):
            W = {}
            W["wh"] = p_ws.tile([128, 4, 2560], BF16, tag="wh", name="wh")
            nc.sync.dma_start(W["wh"][:], wh_d[l])
            W["wl"] = p_ws.tile([128, 4, 3072], BF16, tag="wl", name="wl")
            nc.sync.dma_start(W["wl"][:], wl_d[l])
            W["wr"] = p_ws.tile([128, 4, 3072], BF16, tag="wr", name="wr")
            nc.sync.dma_start(W["wr"][:], wr_d[l])
            return W

        def proj_pass(l, T, direction, chunks, t0=0, t1=None):
            if direction == "b":
                nx, wx_d, bias_t, x2t = 24, wxb_d, T["bbt"], T["x2b"]
            else:
                nx, wx_d, bias_t, x2t = 28, wxf_d, T["bft"], T["x2f"]
            for t in range(t0, nx if t1 is None else t1):
                wt = p_wproj.tile([128, 8, 128], BF16, tag="wproj")
                nc.sync.dma_start(wt[:], wx_d[l, t])
                pss = [p_psp.tile([128, c1 - c0], F32, tag="psp",
                                   name=f"psp{ci}")
                       for ci, (c0, c1) in enumerate(chunks)]
                for k in range(8):
                    for ci, (c0, c1) in enumerate(chunks):
                        nc.tensor.matmul(pss[ci][:], wt[:, k, :],
                                         FB[k][:, c0:c1],
                                         start=(k == 0), stop=(k == 7))
                for ci, (c0, c1) in enumerate(chunks):
                    if ci % 2 == 0:
                        nc.scalar.activation(x2t[:, t, c0:c1], pss[ci][:],
                                             AF.Identity,
                                             bias=bias_t[:, t:t + 1])
                    else:
                        nc.vector.tensor_scalar(x2t[:, t, c0:c1], pss[ci][:],
                                                bias_t[:, t:t + 1], None,
                                                op0=ALU.add)

        # ---------------------------------------------------------- tails
        FUNCS_F = [SIG, SIG, SIG, SIG, SIG, TANH]   # i,o,fl,fr,r,u
        FUNCS_B = [SIG, SIG, SIG, SIG, TANH]        # i,o,f,r,u

        def out_write(m, off, hf, base, out_l, skip0):
            for j in range(4):
                eng = nc.gpsimd if (j % 2) else nc.vector
                eng.tensor_copy(FB[base + j][:, off:off + m], hf[:, j, :])
            if not out_l:
                return
            if off + m <= 134:
                for j in range(4):
                    eng = nc.vector if (j % 2) else nc.gpsimd
                    eng.tensor_copy(ST["hfs"][base + j][:, off:off + m],
                                    hf[:, j, :])
            else:
                s = 1 if skip0 else 0
                if m - s > 0:
                    for j in range(4):
                        nc.gpsimd.dma_start(
                            outT_d[base + j][:, off + s:off + m],
                            hf[:, j, s:m])

        def tail_fw(m, off, gsl, cl_ap, cr_ap, cnew_ap, out_l, skip0=False):
            gi, go, gfl, gfr, gr, gu = (gsl(q) for q in range(6))
            px = ST["x2f"][:, 24:28, off:off + m]
            nc.gpsimd.tensor_tensor(cnew_ap, gi, gu, op=ALU.mult)
            # off-critical: A = o*r, B = (1-r)*px
            A = p_tmp.tile([128, 4, m], BF16, tag="A")
            nc.gpsimd.tensor_tensor(A[:], go, gr, op=ALU.mult)
            B = p_tmp.tile([128, 4, m], BF16, tag="B")
            nc.gpsimd.tensor_tensor(B[:], gr, px, op=ALU.mult)
            nc.gpsimd.tensor_tensor(B[:], px, B[:], op=ALU.subtract)
            if cl_ap is not None:
                ct = p_tmp.tile([128, 4, m], BF16, tag="ct")
                nc.vector.tensor_tensor(ct[:], gfl, cl_ap, op=ALU.mult)
                nc.vector.tensor_tensor(cnew_ap, cnew_ap, ct[:], op=ALU.add)
                if cr_ap is not None:
                    ct2 = p_tmp.tile([128, 4, m], BF16, tag="ct")
                    nc.vector.tensor_tensor(ct2[:], gfr, cr_ap, op=ALU.mult)
                    nc.vector.tensor_tensor(cnew_ap, cnew_ap, ct2[:],
                                            op=ALU.add)
            th = p_tmp.tile([128, 4, m], BF16, tag="th")
            nc.scalar.activation(th[:], cnew_ap, TANH)
            hf = p_tmp.tile([128, 4, m], F32, tag="hf")
            nc.vector.tensor_tensor(hf[:], A[:], th[:], op=ALU.mult)
            nc.vector.tensor_tensor(hf[:], hf[:], B[:], op=ALU.add)
            out_write(m, off, hf, 0, out_l, skip0)
            return hf

        def tail_bw(m, off, gsl, cp_ap, pairs, cnew_ap, out_l):
            gi, go, gf, gr, gu = (gsl(q) for q in range(5))
            px = ST["x2b"][:, 20:24, off:off + m]
            nc.gpsimd.tensor_tensor(cnew_ap, gi, gu, op=ALU.mult)
            A = p_tmp.tile([128, 4, m], BF16, tag="A")
            nc.gpsimd.tensor_tensor(A[:], go, gr, op=ALU.mult)
            B = p_tmp.tile([128, 4, m], BF16, tag="B")
            nc.gpsimd.tensor_tensor(B[:], gr, px, op=ALU.mult)
            nc.gpsimd.tensor_tensor(B[:], px, B[:], op=ALU.subtract)
            if cp_ap is not None:
                ct = p_tmp.tile([128, 4, m], BF16, tag="ct")
                if pairs:
                    nc.vector.tensor_tensor(ct[:, :, 0:m:2],
                                            gf[:, :, 0:m:2], cp_ap,
                                            op=ALU.mult)
                    nc.vector.tensor_tensor(ct[:, :, 1:m:2],
                                            gf[:, :, 1:m:2], cp_ap,
                                            op=ALU.mult)
                else:
                    nc.vector.tensor_tensor(ct[:], gf, cp_ap, op=ALU.mult)
                nc.vector.tensor_tensor(cnew_ap, cnew_ap, ct[:], op=ALU.add)
            th = p_tmp.tile([128, 4, m], BF16, tag="th")
            nc.scalar.activation(th[:], cnew_ap, TANH)
            hf = p_tmp.tile([128, 4, m], F32, tag="hf")
            nc.vector.tensor_tensor(hf[:], A[:], th[:], op=ALU.mult)
            nc.vector.tensor_tensor(hf[:], hf[:], B[:], op=ALU.add)
            out_write(m, off, hf, 4, out_l, False)
            return hf

        # ------------------------------------------------------- fw steps
        def leaf_fw(m, off, cnew_ap, out_l, skip0=False):
            x2f = ST["x2f"]
            if m <= 16:
                A = p_gates.tile([128, 24, m], BF16, tag="g0")
                nc.scalar.activation(A[:, 0:20, :],
                                     x2f[:, 0:20, off:off + m], SIG)
                nc.scalar.activation(A[:, 20:24, :],
                                     x2f[:, 20:24, off:off + m], TANH)
                gsl = lambda q: A[:, 4 * q:4 * q + 4, :]
            else:
                acts = []
                for q in range(6):
                    a = p_gates.tile([128, 4, m], BF16, tag=f"g{q}")
                    nc.scalar.activation(a[:], x2f[:, 4 * q:4 * q + 4,
                                                   off:off + m], FUNCS_F[q])
                    acts.append(a)
                gsl = lambda q: acts[q][:]
            return tail_fw(m, off, gsl, None, None, cnew_ap, out_l, skip0)

        def step_fw(m, off, hl_fn, hr_fn, cl_ap, cr_ap, cnew_ap, out_l,
                    use_wr=True, skip0=False):
            """hl_fn/hr_fn: k -> [128, m] rhs AP (hr_fn None with use_wr False)."""
            x2f = ST["x2f"]
            wl, wr = CUR["wl"], CUR["wr"]
            nmm = 8 if use_wr else 4
            if m <= 16:
                ps = p_psc.tile([128, 24, m], F32, tag="psc")
                for t in range(24):
                    for k in range(4):
                        nc.tensor.matmul(ps[:, t, :],
                                         wl[:, k, 128 * t:128 * (t + 1)],
                                         hl_fn(k), start=(k == 0),
                                         stop=(k == nmm - 1))
                    if use_wr:
                        for k in range(4):
                            nc.tensor.matmul(ps[:, t, :],
                                             wr[:, k, 128 * t:128 * (t + 1)],
                                             hr_fn(k), start=False,
                                             stop=(k == 3))
                g = p_tmp.tile([128, 24, m], F32, tag="g")
                nc.vector.tensor_tensor(g[:], ps[:],
                                        x2f[:, 0:24, off:off + m], op=ALU.add)
                A = p_gates.tile([128, 24, m], BF16, tag="g0")
                nc.scalar.activation(A[:, 0:20, :], g[:, 0:20, :], SIG)
                nc.scalar.activation(A[:, 20:24, :], g[:, 20:24, :], TANH)
                gsl = lambda q: A[:, 4 * q:4 * q + 4, :]
            else:
                acts = []
                for q in range(6):
                    ps = p_pss.tile([128, 4, m], F32, tag="pss")
                    for j4 in range(4):
                        t = 4 * q + j4
                        for k in range(4):
                            nc.tensor.matmul(ps[:, j4, :],
                                             wl[:, k, 128 * t:128 * (t + 1)],
                                             hl_fn(k), start=(k == 0),
                                             stop=(k == nmm - 1))
                        if use_wr:
                            for k in range(4):
                                nc.tensor.matmul(
                                    ps[:, j4, :],
                                    wr[:, k, 128 * t:128 * (t + 1)],
                                    hr_fn(k), start=False, stop=(k == 3))
                    g = p_tmp.tile([128, 4, m], F32, tag="g")
                    nc.vector.tensor_tensor(
                        g[:], ps[:], x2f[:, 4 * q:4 * q + 4, off:off + m],
                        op=ALU.add)
                    a = p_gates.tile([128, 4, m], BF16, tag=f"g{q}")
                    nc.scalar.activation(a[:], g[:], FUNCS_F[q])
                    acts.append(a)
                gsl = lambda q: acts[q][:]
            return tail_fw(m, off, gsl, cl_ap, cr_ap, cnew_ap, out_l, skip0)

        def chunks_of(m):
            # keep whole levels in one call; only bound tile sizes at 128
            return [m] if m <= 128 else [128] * (m // 128)

        def step_fw_lvl(lvl, cc_child, out_l):
            """Generic fw level step reading children from FB / child c tile."""
            m = 1 << (lvl - 3)
            off = _lvl_off(lvl)
            offc = _lvl_off(lvl + 1)
            cn = p_cbuf.tile([128, 4, m], BF16, tag="cfw")
            hf = None
            c0 = 0
            for mc in chunks_of(m):
                o2 = offc + 2 * c0
                hf = step_fw(
                    mc, off + c0,
                    lambda k, o2=o2, mc=mc: FB[k][:, o2:o2 + 2 * mc:2],
                    lambda k, o2=o2, mc=mc: FB[k][:, o2 + 1:o2 + 2 * mc:2],
                    cc_child[:, :, 2 * c0:2 * (c0 + mc):2],
                    cc_child[:, :, 2 * c0 + 1:2 * (c0 + mc):2],
                    cn[:, :, c0:c0 + mc], out_l)
                c0 += mc
            return hf, cn

        # ------------------------------------------------------- bw steps
        def leaf_bw0(out_l):
            """node 0: no parent (zero slot) -> gates from x2 only."""
            x2b = ST["x2b"]
            A = p_gates.tile([128, 20, 1], BF16, tag="g0")
            nc.scalar.activation(A[:, 0:16, :], x2b[:, 0:16, 0:1], SIG)
            nc.scalar.activation(A[:, 16:20, :], x2b[:, 16:20, 0:1], TANH)
            gsl = lambda q: A[:, 4 * q:4 * q + 4, :]
            cn = p_cbuf.tile([128, 4, 1], BF16, tag="cbw")
            hf = tail_bw(1, 0, gsl, None, False, cn[:], out_l)
            return hf, cn

        def step_bw(m, off, hp_fn, mp, cp_ap, cnew_ap, out_l):
            """m kids at cols off.., mp parents; hp_fn: k -> [128, mp] AP."""
            x2b = ST["x2b"]
            wh = CUR["wh"]
            pairs = m > mp
            if mp <= 16:
                ps = p_psc.tile([128, 20, mp], F32, tag="psc")
                for t in range(20):
                    for k in range(4):
                        nc.tensor.matmul(ps[:, t, :],
                                         wh[:, k, 128 * t:128 * (t + 1)],
                                         hp_fn(k), start=(k == 0),
                                         stop=(k == 3))
                g = p_tmp.tile([128, 20, m], F32, tag="g")
                if pairs:
                    nc.vector.tensor_tensor(
                        g[:, :, 0:m:2], ps[:],
                        x2b[:, 0:20, off:off + m:2], op=ALU.add)
                    nc.vector.tensor_tensor(
                        g[:, :, 1:m:2], ps[:],
                        x2b[:, 0:20, off + 1:off + m:2], op=ALU.add)
                else:
                    nc.vector.tensor_tensor(g[:], ps[:],
                                            x2b[:, 0:20, off:off + m],
                                            op=ALU.add)
                A = p_gates.tile([128, 20, m], BF16, tag="g0")
                nc.scalar.activation(A[:, 0:16, :], g[:, 0:16, :], SIG)
                nc.scalar.activation(A[:, 16:20, :], g[:, 16:20, :], TANH)
                gsl = lambda q: A[:, 4 * q:4 * q + 4, :]
            else:
                acts = []
                for q in range(5):
                    ps = p_pss.tile([128, 4, mp], F32, tag="pss")
                    for j4 in range(4):
                        t = 4 * q + j4
                        for k in range(4):
                            nc.tensor.matmul(ps[:, j4, :],
                                             wh[:, k, 128 * t:128 * (t + 1)],
                                             hp_fn(k), start=(k == 0),
                                             stop=(k == 3))
                    g = p_tmp.tile([128, 4, m], F32, tag="g")
                    if pairs:
                        nc.vector.tensor_tensor(
                            g[:, :, 0:m:2], ps[:],
                            x2b[:, 4 * q:4 * q + 4, off:off + m:2],
                            op=ALU.add)
                        nc.vector.tensor_tensor(
                            g[:, :, 1:m:2], ps[:],
                            x2b[:, 4 * q:4 * q + 4, off + 1:off + m:2],
                            op=ALU.add)
                    else:
                        nc.vector.tensor_tensor(
                            g[:], ps[:],
                            x2b[:, 4 * q:4 * q + 4, off:off + m], op=ALU.add)
                    a = p_gates.tile([128, 4, m], BF16, tag=f"g{q}")
                    nc.scalar.activation(a[:], g[:], FUNCS_B[q])
                    acts.append(a)
                gsl = lambda q: acts[q][:]
            return tail_bw(m, off, gsl, cp_ap, pairs, cnew_ap, out_l)

        def step_bw_lvl(lvl, cprev, out_l):
            """Generic bw level step (lvl >= 4): parents at lvl-1 cols."""
            m = 1 << (lvl - 3)
            off = _lvl_off(lvl)
            offp = _lvl_off(lvl - 1)
            cn = p_cbuf.tile([128, 4, m], BF16, tag="cbw")
            hf = None
            c0 = 0
            for mc in chunks_of(m):
                mpc = mc // 2
                hf = step_bw(
                    mc, off + c0,
                    lambda k, p0=offp + c0 // 2, mpc=mpc:
                        FB[4 + k][:, p0:p0 + mpc],
                    mpc, cprev[:, :, c0 // 2:c0 // 2 + mpc],
                    cn[:, :, c0:c0 + mc], out_l)
                c0 += mc
            return hf, cn

        # ------------------------------------------------------- scans
        def scans_emit(l, out_l):
            x2f, x2b = ST["x2f"], ST["x2b"]

            # fw lvl12 leaf (node 4095 slot, col 518)
            c12 = p_small.tile([128, 4, 1], BF16, tag="c12")
            h12 = leaf_fw(1, 518, c12[:], out_l)

            # bw top: nodes 0..6
            h_b0, cb0 = leaf_bw0(out_l)
            cb1 = p_cbuf.tile([128, 4, 2], BF16, tag="cbw")
            h_b1 = step_bw(2, 1, lambda k: FB[4 + k][:, 0:1], 1,
                           cb0[:, :, 0:1], cb1[:], out_l)
            cb2 = p_cbuf.tile([128, 4, 4], BF16, tag="cbw")
            h_b2 = step_bw(4, 3, lambda k: FB[4 + k][:, 1:3], 2,
                           cb1[:, :, 0:2], cb2[:], out_l)

            # bw lvl3 (subtree root, col 7): psel one-hot parent selection.
            # Emitted before the leaf floods so its DVE chain isn't queued
            # behind the leaf tails.
            hps = p_small.tile([128, 4, 1], BF16, tag="hps")
            cps = p_small.tile([128, 4, 1], F32, tag="cps")
            hsel = p_small.tile([128, 4, 1], F32, tag="hsel")
            for j in range(4):
                tsel = p_small.tile([128, 4], F32, tag="tsel")
                nc.vector.tensor_tensor(tsel[:], h_b2[:, j, :], psel_t[:],
                                        op=ALU.mult)
                nc.vector.tensor_reduce(hsel[:, j, :], tsel[:],
                                        mybir.AxisListType.X, ALU.add)
                tsel2 = p_small.tile([128, 4], F32, tag="tsel2")
                nc.vector.tensor_tensor(tsel2[:], cb2[:, j, :], psel_t[:],
                                        op=ALU.mult)
                nc.vector.tensor_reduce(cps[:, j, :], tsel2[:],
                                        mybir.AxisListType.X, ALU.add)
            nc.vector.tensor_copy(hps[:], hsel[:])
            cb3 = p_cbuf.tile([128, 4, 1], BF16, tag="cbw")
            step_bw(1, 7, lambda k: hps[:, k, :], 1, cps[:], cb3[:], out_l)

            # fw lvl11 leaf chunk 1 (cols 262..389) + col-262 correction
            c11 = p_cbuf.tile([128, 4, 256], BF16, tag="cfw")
            leaf_fw(128, 262, c11[:, :, 0:128], out_l, skip0=True)
            hlc = p_small.tile([128, 4, 1], BF16, tag="hlc")
            clc = p_small.tile([128, 4, 1], BF16, tag="clc")
            nc.vector.tensor_scalar(hlc[:], h12[:], lmask_t[:], None,
                                    op0=ALU.mult)
            nc.vector.tensor_scalar(clc[:], c12[:], lmask_t[:], None,
                                    op0=ALU.mult)
            step_fw(1, 262, lambda k: hlc[:, k, :], None, clc[:], None,
                    c11[:, :, 0:1], out_l, use_wr=False)

            # fw lvl11 leaf chunk 2
            leaf_fw(128, 390, c11[:, :, 128:256], out_l)

            st = {"cf": c11, "cb": cb3, "hro": None, "cro": None}

            def emit_bw(lvl):
                if lvl == 12:
                    cn = p_small.tile([128, 4, 1], BF16, tag="c12b")
                    step_bw(1, 518, lambda k: FB[4 + k][:, 262:263], 1,
                            st["cb"][:, :, 0:1], cn[:], out_l)
                    return
                hf, cn = step_bw_lvl(lvl, st["cb"], out_l)
                st["cb"] = cn

            def emit_fw(lvl):
                hf, cn = step_fw_lvl(lvl, st["cf"], out_l)
                st["hro"], st["cf"] = hf, cn

            for blvl, flvl in ((4, 10), (5, 9), (6, 8), (7, 7), (8, 6),
                               (9, 5), (10, 4), (11, 3)):
                emit_bw(blvl)
                emit_fw(flvl)
            emit_bw(12)
            return st["hro"], st["cf"]

        def fwtop_emit(hroots_bf, croots, out_l):
            ct2 = p_small.tile([128, 4, 4], BF16, tag="ct2")
            h2 = step_fw(4, 3,
                         lambda k: hroots_bf[:, k, 0:8:2],
                         lambda k: hroots_bf[:, k, 1:8:2],
                         croots[:, :, 0:8:2], croots[:, :, 1:8:2],
                         ct2[:], out_l)
            ct1 = p_small.tile([128, 4, 2], BF16, tag="ct1")
            h1 = step_fw(2, 1,
                         lambda k: FB[k][:, 3:7:2],
                         lambda k: FB[k][:, 4:7:2],
                         ct2[:, :, 0:4:2], ct2[:, :, 1:4:2],
                         ct1[:], out_l)
            ct0 = p_small.tile([128, 4, 1], BF16, tag="ct0")
            step_fw(1, 0,
                    lambda k: FB[k][:, 1:3:2],
                    lambda k: FB[k][:, 2:3:2],
                    ct1[:, :, 0:1], ct1[:, :, 1:2],
                    ct0[:], out_l)

        # =================================================== layer 0
        T0 = alloc_layer(0)
        ST.update(x2f=T0["x2f"], x2b=T0["x2b"])
        W0 = alloc_scan_weights(0)
        CUR.update(W0)
        proj_pass(0, T0, "b", [(0, 260), (260, 520)])
        proj_pass(0, T0, "f", [(0, 260), (260, 520)])
        # snapshot the fw head (cols 0..6) so fwtop doesn't pin the x2f buffer
        hd0 = p_small.tile([128, 28, 7], BF16, tag="hd0")
        nc.vector.tensor_copy(hd0[:], T0["x2f"][:, :, 0:7])
        hro0, cf0 = scans_emit(0, False)

        # allgather subtree roots of layer 0
        ccin = p_dram.tile([1024], F32, tag="ccin")
        ccout = p_dram.tile([NCORES, 1024], F32, tag="ccout")
        ccin_v = ccin[:].rearrange("(j p) -> p j", p=128)
        nc.gpsimd.dma_start(ccin_v[:, 0:4], hro0[:, :, 0])
        cfs = p_small.tile([128, 4], F32, tag="cfs")
        nc.vector.tensor_copy(cfs[:], cf0[:, :, 0])
        nc.gpsimd.dma_start(ccin_v[:, 4:8], cfs[:])
        nc.gpsimd.collective_compute(
            "AllGather", mybir.AluOpType.bypass,
            replica_groups=[list(range(NCORES))],
            ins=[ccin[:].opt()], outs=[ccout[:].opt()])
        ccout_v = ccout[:].rearrange("r (j p) -> p j r", p=128)
        hroots = p_small.tile([128, 4, 8], F32, tag="hroots")
        croots = p_small.tile([128, 4, 8], F32, tag="croots")
        for j in range(4):
            nc.gpsimd.dma_start(hroots[:, j, :], ccout_v[:, j, :])
            nc.gpsimd.dma_start(croots[:, j, :], ccout_v[:, 4 + j, :])
        hroots_bf = p_small.tile([128, 4, 8], BF16, tag="hrootsb")
        nc.vector.tensor_copy(hroots_bf[:], hroots[:])

        # =================================================== layer 1
        T1 = alloc_layer(1)
        ST.update(x2f=T1["x2f"], x2b=T1["x2b"])
        hfs = [p_hfs.tile([128, 134], F32, tag=f"hfs{j}", name=f"hfs{j}")
               for j in range(8)]
        ST["hfs"] = hfs

        # main passes (subtree cols; overlap the collective)
        proj_pass(1, T1, "b", [(7, 264), (264, 520)])
        proj_pass(1, T1, "f", [(7, 264), (264, 520)])

        # finish layer 0: redundant top-7 fw scan (layer-0 weights, head x2)
        ST.update(x2f=hd0)
        fwtop_emit(hroots_bf, croots, False)
        ST.update(x2f=T1["x2f"])

        # bw head pass (cols 0..6 need fwtop-0)
        proj_pass(1, T1, "b", [(0, 7)])
        for j in range(8):
            nc.sync.dma_start(f2top_d[j], FB[j][:, 0:7])

        W1 = alloc_scan_weights(1)
        CUR.update(W1)
        hro1, cf1 = scans_emit(1, True)
        rcs = p_small.tile([128, 4], F32, tag="rcs")
        nc.vector.tensor_copy(rcs[:], cf1[:, :, 0])
        nc.sync.dma_start(rootc_d[:], rcs[:])
        for j in range(8):
            nc.gpsimd.dma_start(outT_d[j][:, 0:134], hfs[j][:])

    nc.compile()
    return nc


# ------------------------------------------------------------------ packing
def _pack_inputs(inp):
    def bf(x):
        return np.ascontiguousarray(x).astype(BF16NP)

    feats = np.asarray(inp["features"], np.float32)
    per_core = []
    wl = np.stack([np.asarray(inp["fw_Wl"][l], np.float32)[_PF].T
                   .reshape(4, 128, 3072).transpose(1, 0, 2) for l in range(L)])
    wr = np.stack([np.asarray(inp["fw_Wr"][l], np.float32)[_PF].T
                   .reshape(4, 128, 3072).transpose(1, 0, 2) for l in range(L)])
    wh = np.stack([np.asarray(inp["bw_Wh"][l], np.float32)[_PB].T
                   .reshape(4, 128, 2560).transpose(1, 0, 2) for l in range(L)])

    def proj_pack(w):
        # w = W.T [1024, M] -> [M/128, 128p, 8k, 128m]
        M = w.shape[1]
        v = w.reshape(8, 128, M // 128, 128)  # (k, p, t, m)
        return np.ascontiguousarray(v.transpose(2, 1, 0, 3))

    wxf = np.stack([
        proj_pack(np.concatenate(
            [np.asarray(inp["fw_Wx"][l], np.float32)[_PF],
             np.asarray(inp["fw_Wp"][l], np.float32)], 0).T)
        for l in range(L)])
    wxb = np.stack([
        proj_pack(np.concatenate(
            [np.asarray(inp["bw_Wx"][l], np.float32)[_PB],
             np.asarray(inp["bw_Wp"][l], np.float32)], 0).T)
        for l in range(L)])
    bfv = np.stack([
        np.concatenate([
            (np.asarray(inp["fw_bx"][l], np.float32)
             + np.asarray(inp["fw_bl"][l], np.float32)
             + np.asarray(inp["fw_br"][l], np.float32))[_PF],
            np.asarray(inp["fw_bp"][l], np.float32)], 0)
        .reshape(28, 128).T for l in range(L)])
    bbv = np.stack([
        np.concatenate([
            (np.asarray(inp["bw_bx"][l], np.float32)
             + np.asarray(inp["bw_bh"][l], np.float32))[_PB],
            np.asarray(inp["bw_bp"][l], np.float32)], 0)
        .reshape(24, 128).T for l in range(L)])
    base = {
        "wl": bf(wl), "wr": bf(wr), "wh": bf(wh),
        "wxf": bf(wxf), "wxb": bf(wxb),
        "bf": np.ascontiguousarray(bfv, dtype=np.float32),
        "bb": np.ascontiguousarray(bbv, dtype=np.float32),
    }
    for c in range(NCORES):
        cm = _col_map(c)
        v = cm >= 0
        fT = np.zeros((1024, NC), np.float32)
        fT[:, v] = feats[cm[v]].T
        psel = np.zeros((128, 4), np.float32)
        psel[:, c // 2] = 1.0
        lmask = np.full((128, 1), 1.0 if c == 0 else 0.0, np.float32)
        m = dict(base)
        m["featsT"] = bf(fT.reshape(8, 128, NC))
        m["psel"] = psel
        m["lmask"] = lmask
        per_core.append(m)
    return per_core


def _host_fwtop(inp, results):
    """Compute the final layer's top-7 forward h on the host, mirroring the
    device arithmetic (bf16 matmul inputs, fp32 accumulation)."""
    l = L - 1

    def bf(x):
        return x.astype(BF16NP).astype(np.float32)

    def sig(x):
        return 1.0 / (1.0 + np.exp(-x))

    # features of layer 1 at nodes 0..6 (bf16 as on device)
    f2 = np.concatenate([np.asarray(results[0]["f2top"], np.float32)[j]
                         for j in range(8)], 0)  # [1024, 7]
    wxf = np.concatenate([np.asarray(inp["fw_Wx"][l], np.float32),
                          np.asarray(inp["fw_Wp"][l], np.float32)], 0)
    bxf = np.concatenate([
        np.asarray(inp["fw_bx"][l], np.float32)
        + np.asarray(inp["fw_bl"][l], np.float32)
        + np.asarray(inp["fw_br"][l], np.float32),
        np.asarray(inp["fw_bp"][l], np.float32)], 0)
    pf = bf(wxf) @ f2 + bxf[:, None]  # f2 already bf16-rounded
    x2, px = pf[:3072], pf[3072:]
    wl = bf(np.asarray(inp["fw_Wl"][l], np.float32))
    wr = bf(np.asarray(inp["fw_Wr"][l], np.float32))

    # subtree roots: h from outT col 7 (f32), c from rootc
    hr8 = np.stack([np.asarray(results[c]["outT"], np.float32)
                    .reshape(1024, NC)[0:512, 7] for c in range(NCORES)], 1)
    cr8 = np.stack([np.asarray(results[c]["rootc"], np.float32)
                    .T.reshape(512) for c in range(NCORES)], 1)

    hbuf = np.zeros((512, 7), np.float32)
    cbuf = np.zeros((512, 7), np.float32)

    def step(cols, hl, hr, cl, cr):
        g = x2[:, cols] + wl @ bf(hl) + wr @ bf(hr)
        i_, o, fl, fr, u, r = (g[k * 512:(k + 1) * 512] for k in range(6))
        i_, o, fl, fr, r = sig(i_), sig(o), sig(fl), sig(fr), sig(r)
        u = np.tanh(u)
        cc = i_ * u + fl * cl + fr * cr
        hc = o * np.tanh(cc)
        return cc, hc * r + (1.0 - r) * px[:, cols]

    cc, hf = step([3, 4, 5, 6], hr8[:, 0::2], hr8[:, 1::2],
                  cr8[:, 0::2], cr8[:, 1::2])
    hbuf[:, 3:7], cbuf[:, 3:7] = hf, cc
    cc, hf = step([1, 2], hbuf[:, 3:7:2], hbuf[:, 4:7:2],
                  cbuf[:, 3:7:2], cbuf[:, 4:7:2])
    hbuf[:, 1:3], cbuf[:, 1:3] = hf, cc
    cc, hf = step([0], hbuf[:, 1:2], hbuf[:, 2:3],
                  cbuf[:, 1:2], cbuf[:, 2:3])
    hbuf[:, 0:1] = hf
    return hbuf  # [512, 7]


def _assemble(inp, results):
    out = np.zeros((N, 1024), np.float32)
    for c in range(NCORES):
        cm = _col_map(c)
        o = np.asarray(results[c]["outT"]).reshape(1024, NC)
        cols = np.arange(NC)
        use = (cm >= 0) & (cols >= 7) & (cols != 519)
        if c != 0:
            use &= cols != 518
        out[cm[use]] = o[:, use].T
        if c == 0:
            out[0:7, 512:1024] = o[512:1024, 0:7].T  # bw half from device
    out[0:7, 0:512] = _host_fwtop(inp, results).T
    return out


def kernel(**inputs):
    inp = {k: np.asarray(v) for k, v in inputs.items()}
    if not _is_canonical(inp):
        return _fallback(inp)
    if "nc" not in _CACHE:
        _CACHE["nc"] = _build_nc()
    from concourse.bass_utils import run_bass_kernel_spmd

    in_maps = _pack_inputs(inp)
    res = run_bass_kernel_spmd(_CACHE["nc"], in_maps,
                               core_ids=list(range(NCORES)))
    return _assemble(inp, res.results)


if __name__ == "__main__":
    d = np.load("/tmp/inputs.npz")
    inputs = {k: d[k] for k in d.files}
    expected = np.load("/tmp/expected.npy")
    actual = kernel(**inputs)
    err = np.abs(actual - expected)
    print("max abs err:", err.max())
    print("absmax-rel:", err.max() / np.abs(expected).max())
    print("mean abs:", err.mean())


# revision 4
# speedup vs baseline: 1.1824x; 1.0208x over previous
"""Trainium2 Bass kernel for MultiLayer bidirectional BTreeLSTM (4096-node
balanced heap tree, IN=OUT=1024, H=512, L=2).

v2 strategy (vs v1 baseline):
- x2/px projections stay SBUF-resident in bf16 (no DRAM round-trip), one
  weight pass per layer with two 260-col matmul chunks per LDWEIGHTS.
- Scan matmuls read h directly from the persistent FB tiles via strided
  APs (no staging copies); the backward direction multiplies over the
  m/2 parent columns only (both children share W@h_parent) and the
  per-child x2 add re-expands via strided adds.
- Gate blocks are reordered host-side to [i,o,fl,fr,r,u] (fw) and
  [i,o,f,r,u] (bw) so small levels (m<=16) run as one PSUM tile with a
  single gate-sum add and just two activation instructions.
- Small-level outputs accumulate in an f32 SBUF strip (cols 0..133) and
  ship to DRAM in one DMA per 128-row block.

Per-core column layout (NC=520): [0..6]=nodes 0..6, [7..517]=subtree
levels 3..11 in level order, [518]=level-12 slot (node 4095, core 0 only),
[519]=pad.
"""

import numpy as np
import ml_dtypes

F8NP = ml_dtypes.float8_e4m3fn
N = 4096
H = 512
L = 2
NCORES = 8
NC = 520
BF16NP = ml_dtypes.bfloat16

_CACHE = {}

# gate-block permutations (rows of the 6H/5H gate dim)
_PF = np.concatenate([np.arange(512 * b, 512 * (b + 1))
                      for b in (0, 1, 2, 3, 5, 4)])  # i,o,fl,fr,r,u
_PB = np.concatenate([np.arange(512 * b, 512 * (b + 1))
                      for b in (0, 1, 2, 4, 3)])     # i,o,f,r,u


# ----------------------------------------------------------------- host utils
def _lvl_off(lvl):
    return 7 + (1 << (lvl - 3)) - 1


def _col_map(core):
    r = 7 + core
    ids = list(range(7))
    for lvl in range(3, 12):
        w = 1 << (lvl - 3)
        start = (r + 1) * w - 1
        ids.extend(range(start, start + w))
    ids.append(4095 if core == 0 else -1)
    ids.append(-1)
    return np.array(ids, np.int64)


def _is_canonical(inp):
    n = N
    i = np.arange(n)
    left = np.where(2 * i + 1 < n, 2 * i + 1, n).astype(np.int32)
    right = np.where(2 * i + 2 < n, 2 * i + 2, n).astype(np.int32)
    parent = np.where(i > 0, (i - 1) // 2, n).astype(np.int32)
    if inp["features"].shape != (N, 1024):
        return False
    for k, v in (("left_child", left), ("right_child", right), ("parent", parent)):
        if inp[k].shape != (n,) or not np.array_equal(np.asarray(inp[k]), v):
            return False
    po = np.asarray(inp["post_order"])
    pr = np.asarray(inp["pre_order"])
    if sorted(po.tolist()) != list(range(n)) or sorted(pr.tolist()) != list(range(n)):
        return False
    pos = np.empty(n, np.int64)
    pos[po] = np.arange(n)
    ok = True
    for child in (left, right):
        m = child < n
        ok &= bool((pos[i[m]] > pos[child[m]]).all())
    pos[pr] = np.arange(n)
    m = parent < n
    ok &= bool((pos[i[m]] > pos[parent[m]]).all())
    return ok


def _fallback(inp):
    """Literal numpy re-implementation of the reference scan (any inputs)."""
    f = {k: np.asarray(v) for k, v in inp.items()}
    feats = f["features"].astype(np.float32)
    n = feats.shape[0]

    def sig(x):
        return 1.0 / (1.0 + np.exp(-x))

    for l in range(L):
        h = f["fw_bp"][l].shape[0]
        px = feats @ f["fw_Wp"][l].T + f["fw_bp"][l]
        x2 = feats @ f["fw_Wx"][l].T + f["fw_bx"][l]
        cbuf = np.zeros((n + 1, h), np.float32)
        hbuf = np.zeros((n + 1, h), np.float32)
        Wl, bl, Wr, br = f["fw_Wl"][l], f["fw_bl"][l], f["fw_Wr"][l], f["fw_br"][l]
        for idx in f["post_order"]:
            lc, rc = f["left_child"][idx], f["right_child"][idx]
            g = x2[idx] + hbuf[lc] @ Wl.T + bl + hbuf[rc] @ Wr.T + br
            i_, o, fl, fr, u, r = np.split(g, 6)
            i_, o, fl, fr, r = sig(i_), sig(o), sig(fl), sig(fr), sig(r)
            u = np.tanh(u)
            c = i_ * u + fl * cbuf[lc] + fr * cbuf[rc]
            hc = o * np.tanh(c)
            cbuf[idx] = c
            hbuf[idx] = r * hc + (1.0 - r) * px[idx]
        h_fwd = hbuf[:n].copy()

        px = feats @ f["bw_Wp"][l].T + f["bw_bp"][l]
        x2 = feats @ f["bw_Wx"][l].T + f["bw_bx"][l]
        cbuf = np.zeros((n + 1, h), np.float32)
        hbuf = np.zeros((n + 1, h), np.float32)
        Wh, bh = f["bw_Wh"][l], f["bw_bh"][l]
        for idx in f["pre_order"]:
            p = f["parent"][idx]
            g = x2[idx] + hbuf[p] @ Wh.T + bh
            i_, o, fo, u, r = np.split(g, 5)
            i_, o, fo, r = sig(i_), sig(o), sig(fo), sig(r)
            u = np.tanh(u)
            c = i_ * u + fo * cbuf[p]
            hc = o * np.tanh(c)
            cbuf[idx] = c
            hbuf[idx] = r * hc + (1.0 - r) * px[idx]
        h_bwd = hbuf[:n].copy()
        feats = np.concatenate([h_fwd, h_bwd], axis=1)
    return feats


# ------------------------------------------------------------- bass program
def _build_nc():
    from contextlib import ExitStack

    import concourse.bacc as bacc
    import concourse.mybir as mybir
    import concourse.tile as tile

    F32 = mybir.dt.float32
    BF16 = mybir.dt.bfloat16
    F8 = mybir.dt.float8e4
    AF = mybir.ActivationFunctionType
    ALU = mybir.AluOpType
    SIG = AF.Sigmoid
    TANH = AF.Tanh

    nc = bacc.Bacc("TRN2", target_bir_lowering=False, debug=False,
                   num_devices=NCORES)

    featsT_d = nc.dram_tensor("featsT", [8, 128, NC], BF16, kind="ExternalInput")
    wl_d = nc.dram_tensor("wl", [L, 128, 4, 3072], BF16, kind="ExternalInput")
    wr_d = nc.dram_tensor("wr", [L, 128, 4, 3072], BF16, kind="ExternalInput")
    wh_d = nc.dram_tensor("wh", [L, 128, 4, 2560], BF16, kind="ExternalInput")
    wxf_d = nc.dram_tensor("wxf", [L, 28, 128, 8, 128], BF16,
                           kind="ExternalInput")
    wxb_d = nc.dram_tensor("wxb", [L, 24, 128, 8, 128], BF16,
                           kind="ExternalInput")
    bf_d = nc.dram_tensor("bf", [L, 128, 28], F32, kind="ExternalInput")
    bb_d = nc.dram_tensor("bb", [L, 128, 24], F32, kind="ExternalInput")
    psel_d = nc.dram_tensor("psel", [128, 4], F32, kind="ExternalInput")
    lmask_d = nc.dram_tensor("lmask", [128, 1], F32, kind="ExternalInput")
    outT_d = nc.dram_tensor("outT", [8, 128, NC], F32, kind="ExternalOutput")
    rootc_d = nc.dram_tensor("rootc", [128, 4], F32, kind="ExternalOutput")
    f2top_d = nc.dram_tensor("f2top", [8, 128, 7], BF16, kind="ExternalOutput")

    with ExitStack() as ctx:
        tc = ctx.enter_context(tile.TileContext(nc))

        p_fb = ctx.enter_context(tc.tile_pool(name="fb", bufs=1))
        p_x2 = ctx.enter_context(tc.tile_pool(name="x2", bufs=1))
        p_ws = ctx.enter_context(tc.tile_pool(name="ws", bufs=1))
        p_wproj = ctx.enter_context(tc.tile_pool(name="wproj", bufs=6))
        p_bias = ctx.enter_context(tc.tile_pool(name="bias", bufs=2))
        p_gates = ctx.enter_context(tc.tile_pool(name="gates", bufs=3))
        p_tmp = ctx.enter_context(tc.tile_pool(name="tmp", bufs=3))
        p_cbuf = ctx.enter_context(tc.tile_pool(name="cbuf", bufs=2))
        p_small = ctx.enter_context(tc.tile_pool(name="small", bufs=2))
        p_hfs = ctx.enter_context(tc.tile_pool(name="hfs", bufs=1))
        p_psp = ctx.enter_context(tc.tile_pool(name="psp", bufs=2, space="PSUM"))
        p_pss = ctx.enter_context(tc.tile_pool(name="pss", bufs=4, space="PSUM"))
        p_psc = ctx.enter_context(tc.tile_pool(name="psc", bufs=2, space="PSUM"))
        p_dram = ctx.enter_context(tc.tile_pool(name="dram", bufs=2, space="DRAM"))

        # persistent feature/h storage (bf16): rows 128j..128j+127
        FB = []
        for j in range(8):
            t = p_fb.tile([128, NC], BF16, tag=f"fb{j}")
            nc.sync.dma_start(t[:], featsT_d[j])
            FB.append(t)
        psel_t = p_small.tile([128, 4], F32, tag="psel")
        nc.sync.dma_start(psel_t[:], psel_d[:])
        lmask_t = p_small.tile([128, 1], F32, tag="lmask")
        nc.sync.dma_start(lmask_t[:], lmask_d[:])

        CUR = {}   # current layer's scan weights
        ST = {}    # scan state: x2f/x2b tiles, hfs tiles, etc.

        # ---------------------------------------------------------- proj
        def alloc_layer(l):
            T = {}
            T["bft"] = p_bias.tile([128, 28], F32, tag="bf", name="bft")
            nc.sync.dma_start(T["bft"][:], bf_d[l])
            T["bbt"] = p_bias.tile([128, 24], F32, tag="bb", name="bbt")
            nc.sync.dma_start(T["bbt"][:], bb_d[l])
            T["x2f"] = p_x2.tile([128, 28, NC], BF16, tag="x2f", name="x2f")
            T["x2b"] = p_x2.tile([128, 24, NC], BF16, tag="x2b", name="x2b")
            return T

        def alloc_scan_weights(l):
            W = {}
            W["wh"] = p_ws.tile([128, 4, 2560], BF16, tag="wh", name="wh")
            nc.sync.dma_start(W["wh"][:], wh_d[l])
            W["wl"] = p_ws.tile([128, 4, 3072], BF16, tag="wl", name="wl")
            nc.sync.dma_start(W["wl"][:], wl_d[l])
            W["wr"] = p_ws.tile([128, 4, 3072], BF16, tag="wr", name="wr")
            nc.sync.dma_start(W["wr"][:], wr_d[l])
            return W

        def proj_pass(l, T, direction, chunks, t0=0, t1=None):
            if direction == "b":
                nx, wx_d, bias_t, x2t = 24, wxb_d, T["bbt"], T["x2b"]
            else:
                nx, wx_d, bias_t, x2t = 28, wxf_d, T["bft"], T["x2f"]
            for t in range(t0, nx if t1 is None else t1):
                wt = p_wproj.tile([128, 8, 128], BF16, tag="wproj")
                nc.sync.dma_start(wt[:], wx_d[l, t])
                pss = [p_psp.tile([128, c1 - c0], F32, tag="psp",
                                   name=f"psp{ci}")
                       for ci, (c0, c1) in enumerate(chunks)]
                for k in range(8):
                    for ci, (c0, c1) in enumerate(chunks):
                        nc.tensor.matmul(pss[ci][:], wt[:, k, :],
                                         FB[k][:, c0:c1],
                                         start=(k == 0), stop=(k == 7))
                for ci, (c0, c1) in enumerate(chunks):
                    if ci % 2 == 0:
                        nc.scalar.activation(x2t[:, t, c0:c1], pss[ci][:],
                                             AF.Identity,
                                             bias=bias_t[:, t:t + 1])
                    else:
                        nc.vector.tensor_scalar(x2t[:, t, c0:c1], pss[ci][:],
                                                bias_t[:, t:t + 1], None,
                                                op0=ALU.add)

        # ---------------------------------------------------------- tails
        FUNCS_F = [SIG, SIG, SIG, SIG, SIG, TANH]   # i,o,fl,fr,r,u
        FUNCS_B = [SIG, SIG, SIG, SIG, TANH]        # i,o,f,r,u

        def out_write(m, off, hf, base, out_l, skip0):
            for j in range(4):
                eng = nc.gpsimd if (j % 2) else nc.vector
                eng.tensor_copy(FB[base + j][:, off:off + m], hf[:, j, :])
            if not out_l:
                return
            if off + m <= 134:
                for j in range(4):
                    eng = nc.vector if (j % 2) else nc.gpsimd
                    eng.tensor_copy(ST["hfs"][base + j][:, off:off + m],
                                    hf[:, j, :])
            else:
                s = 1 if skip0 else 0
                if m - s > 0:
                    for j in range(4):
                        nc.gpsimd.dma_start(
                            outT_d[base + j][:, off + s:off + m],
                            hf[:, j, s:m])

        def tail_fw(m, off, gsl, cl_ap, cr_ap, cnew_ap, out_l, skip0=False):
            gi, go, gfl, gfr, gr, gu = (gsl(q) for q in range(6))
            px = ST["x2f"][:, 24:28, off:off + m]
            nc.gpsimd.tensor_tensor(cnew_ap, gi, gu, op=ALU.mult)
            # off-critical: A = o*r, B = (1-r)*px
            A = p_tmp.tile([128, 4, m], BF16, tag="A")
            nc.gpsimd.tensor_tensor(A[:], go, gr, op=ALU.mult)
            B = p_tmp.tile([128, 4, m], BF16, tag="B")
            nc.gpsimd.tensor_tensor(B[:], gr, px, op=ALU.mult)
            nc.gpsimd.tensor_tensor(B[:], px, B[:], op=ALU.subtract)
            if cl_ap is not None:
                ct = p_tmp.tile([128, 4, m], BF16, tag="ct")
                nc.vector.tensor_tensor(ct[:], gfl, cl_ap, op=ALU.mult)
                nc.vector.tensor_tensor(cnew_ap, cnew_ap, ct[:], op=ALU.add)
                if cr_ap is not None:
                    ct2 = p_tmp.tile([128, 4, m], BF16, tag="ct")
                    nc.vector.tensor_tensor(ct2[:], gfr, cr_ap, op=ALU.mult)
                    nc.vector.tensor_tensor(cnew_ap, cnew_ap, ct2[:],
                                            op=ALU.add)
            th = p_tmp.tile([128, 4, m], BF16, tag="th")
            nc.scalar.activation(th[:], cnew_ap, TANH)
            hf = p_tmp.tile([128, 4, m], F32, tag="hf")
            nc.vector.tensor_tensor(hf[:], A[:], th[:], op=ALU.mult)
            nc.vector.tensor_tensor(hf[:], hf[:], B[:], op=ALU.add)
            out_write(m, off, hf, 0, out_l, skip0)
            return hf

        def tail_bw(m, off, gsl, cp_ap, pairs, cnew_ap, out_l):
            gi, go, gf, gr, gu = (gsl(q) for q in range(5))
            px = ST["x2b"][:, 20:24, off:off + m]
            nc.gpsimd.tensor_tensor(cnew_ap, gi, gu, op=ALU.mult)
            A = p_tmp.tile([128, 4, m], BF16, tag="A")
            nc.gpsimd.tensor_tensor(A[:], go, gr, op=ALU.mult)
            B = p_tmp.tile([128, 4, m], BF16, tag="B")
            nc.gpsimd.tensor_tensor(B[:], gr, px, op=ALU.mult)
            nc.gpsimd.tensor_tensor(B[:], px, B[:], op=ALU.subtract)
            if cp_ap is not None:
                ct = p_tmp.tile([128, 4, m], BF16, tag="ct")
                if pairs:
                    nc.vector.tensor_tensor(ct[:, :, 0:m:2],
                                            gf[:, :, 0:m:2], cp_ap,
                                            op=ALU.mult)
                    nc.vector.tensor_tensor(ct[:, :, 1:m:2],
                                            gf[:, :, 1:m:2], cp_ap,
                                            op=ALU.mult)
                else:
                    nc.vector.tensor_tensor(ct[:], gf, cp_ap, op=ALU.mult)
                nc.vector.tensor_tensor(cnew_ap, cnew_ap, ct[:], op=ALU.add)
            th = p_tmp.tile([128, 4, m], BF16, tag="th")
            nc.scalar.activation(th[:], cnew_ap, TANH)
            hf = p_tmp.tile([128, 4, m], F32, tag="hf")
            nc.vector.tensor_tensor(hf[:], A[:], th[:], op=ALU.mult)
            nc.vector.tensor_tensor(hf[:], hf[:], B[:], op=ALU.add)
            out_write(m, off, hf, 4, out_l, False)
            return hf

        # ------------------------------------------------------- fw steps
        def leaf_fw(m, off, cnew_ap, out_l, skip0=False):
            x2f = ST["x2f"]
            if m <= 16:
                A = p_gates.tile([128, 24, m], BF16, tag="g0")
                nc.scalar.activation(A[:, 0:20, :],
                                     x2f[:, 0:20, off:off + m], SIG)
                nc.scalar.activation(A[:, 20:24, :],
                                     x2f[:, 20:24, off:off + m], TANH)
                gsl = lambda q: A[:, 4 * q:4 * q + 4, :]
            else:
                acts = []
                for q in range(6):
                    a = p_gates.tile([128, 4, m], BF16, tag=f"g{q}")
                    nc.scalar.activation(a[:], x2f[:, 4 * q:4 * q + 4,
                                                   off:off + m], FUNCS_F[q])
                    acts.append(a)
                gsl = lambda q: acts[q][:]
            return tail_fw(m, off, gsl, None, None, cnew_ap, out_l, skip0)

        def step_fw(m, off, hl_fn, hr_fn, cl_ap, cr_ap, cnew_ap, out_l,
                    use_wr=True, skip0=False):
            """hl_fn/hr_fn: k -> [128, m] rhs AP (hr_fn None with use_wr False)."""
            x2f = ST["x2f"]
            wl, wr = CUR["wl"], CUR["wr"]
            nmm = 8 if use_wr else 4
            if m <= 16:
                ps = p_psc.tile([128, 24, m], F32, tag="psc")
                for t in range(24):
                    for k in range(4):
                        nc.tensor.matmul(ps[:, t, :],
                                         wl[:, k, 128 * t:128 * (t + 1)],
                                         hl_fn(k), start=(k == 0),
                                         stop=(k == nmm - 1))
                    if use_wr:
                        for k in range(4):
                            nc.tensor.matmul(ps[:, t, :],
                                             wr[:, k, 128 * t:128 * (t + 1)],
                                             hr_fn(k), start=False,
                                             stop=(k == 3))
                g = p_tmp.tile([128, 24, m], F32, tag="g")
                nc.vector.tensor_tensor(g[:], ps[:],
                                        x2f[:, 0:24, off:off + m], op=ALU.add)
                A = p_gates.tile([128, 24, m], BF16, tag="g0")
                nc.scalar.activation(A[:, 0:20, :], g[:, 0:20, :], SIG)
                nc.scalar.activation(A[:, 20:24, :], g[:, 20:24, :], TANH)
                gsl = lambda q: A[:, 4 * q:4 * q + 4, :]
            else:
                acts = []
                for q in range(6):
                    ps = p_pss.tile([128, 4, m], F32, tag="pss")
                    for j4 in range(4):
                        t = 4 * q + j4
                        for k in range(4):
                            nc.tensor.matmul(ps[:, j4, :],
                                             wl[:, k, 128 * t:128 * (t + 1)],
                                             hl_fn(k), start=(k == 0),
                                             stop=(k == nmm - 1))
                        if use_wr:
                            for k in range(4):
                                nc.tensor.matmul(
                                    ps[:, j4, :],
                                    wr[:, k, 128 * t:128 * (t + 1)],
                                    hr_fn(k), start=False, stop=(k == 3))
                    g = p_tmp.tile([128, 4, m], F32, tag="g")
                    nc.vector.tensor_tensor(
                        g[:], ps[:], x2f[:, 4 * q:4 * q + 4, off:off + m],
                        op=ALU.add)
                    a = p_gates.tile([128, 4, m], BF16, tag=f"g{q}")
                    nc.scalar.activation(a[:], g[:], FUNCS_F[q])
                    acts.append(a)
                gsl = lambda q: acts[q][:]
            return tail_fw(m, off, gsl, cl_ap, cr_ap, cnew_ap, out_l, skip0)

        def chunks_of(m):
            # keep whole levels in one call; only bound tile sizes at 128
            return [m] if m <= 128 else [128] * (m // 128)

        def step_fw_lvl(lvl, cc_child, out_l):
            """Generic fw level step reading children from FB / child c tile."""
            m = 1 << (lvl - 3)
            off = _lvl_off(lvl)
            offc = _lvl_off(lvl + 1)
            cn = p_cbuf.tile([128, 4, m], BF16, tag="cfw")
            hf = None
            c0 = 0
            for mc in chunks_of(m):
                o2 = offc + 2 * c0
                hf = step_fw(
                    mc, off + c0,
                    lambda k, o2=o2, mc=mc: FB[k][:, o2:o2 + 2 * mc:2],
                    lambda k, o2=o2, mc=mc: FB[k][:, o2 + 1:o2 + 2 * mc:2],
                    cc_child[:, :, 2 * c0:2 * (c0 + mc):2],
                    cc_child[:, :, 2 * c0 + 1:2 * (c0 + mc):2],
                    cn[:, :, c0:c0 + mc], out_l)
                c0 += mc
            return hf, cn

        # ------------------------------------------------------- bw steps
        def leaf_bw0(out_l):
            """node 0: no parent (zero slot) -> gates from x2 only."""
            x2b = ST["x2b"]
            A = p_gates.tile([128, 20, 1], BF16, tag="g0")
            nc.scalar.activation(A[:, 0:16, :], x2b[:, 0:16, 0:1], SIG)
            nc.scalar.activation(A[:, 16:20, :], x2b[:, 16:20, 0:1], TANH)
            gsl = lambda q: A[:, 4 * q:4 * q + 4, :]
            cn = p_cbuf.tile([128, 4, 1], BF16, tag="cbw")
            hf = tail_bw(1, 0, gsl, None, False, cn[:], out_l)
            return hf, cn

        def step_bw(m, off, hp_fn, mp, cp_ap, cnew_ap, out_l):
            """m kids at cols off.., mp parents; hp_fn: k -> [128, mp] AP."""
            x2b = ST["x2b"]
            wh = CUR["wh"]
            pairs = m > mp
            if mp <= 16:
                ps = p_psc.tile([128, 20, mp], F32, tag="psc")
                for t in range(20):
                    for k in range(4):
                        nc.tensor.matmul(ps[:, t, :],
                                         wh[:, k, 128 * t:128 * (t + 1)],
                                         hp_fn(k), start=(k == 0),
                                         stop=(k == 3))
                g = p_tmp.tile([128, 20, m], F32, tag="g")
                if pairs:
                    nc.vector.tensor_tensor(
                        g[:, :, 0:m:2], ps[:],
                        x2b[:, 0:20, off:off + m:2], op=ALU.add)
                    nc.vector.tensor_tensor(
                        g[:, :, 1:m:2], ps[:],
                        x2b[:, 0:20, off + 1:off + m:2], op=ALU.add)
                else:
                    nc.vector.tensor_tensor(g[:], ps[:],
                                            x2b[:, 0:20, off:off + m],
                                            op=ALU.add)
                A = p_gates.tile([128, 20, m], BF16, tag="g0")
                nc.scalar.activation(A[:, 0:16, :], g[:, 0:16, :], SIG)
                nc.scalar.activation(A[:, 16:20, :], g[:, 16:20, :], TANH)
                gsl = lambda q: A[:, 4 * q:4 * q + 4, :]
            else:
                acts = []
                for q in range(5):
                    ps = p_pss.tile([128, 4, mp], F32, tag="pss")
                    for j4 in range(4):
                        t = 4 * q + j4
                        for k in range(4):
                            nc.tensor.matmul(ps[:, j4, :],
                                             wh[:, k, 128 * t:128 * (t + 1)],
                                             hp_fn(k), start=(k == 0),
                                             stop=(k == 3))
                    g = p_tmp.tile([128, 4, m], F32, tag="g")
                    if pairs:
                        nc.vector.tensor_tensor(
                            g[:, :, 0:m:2], ps[:],
                            x2b[:, 4 * q:4 * q + 4, off:off + m:2],
                            op=ALU.add)
                        nc.vector.tensor_tensor(
                            g[:, :, 1:m:2], ps[:],
                            x2b[:, 4 * q:4 * q + 4, off + 1:off + m:2],
                            op=ALU.add)
                    else:
                        nc.vector.tensor_tensor(
                            g[:], ps[:],
                            x2b[:, 4 * q:4 * q + 4, off:off + m], op=ALU.add)
                    a = p_gates.tile([128, 4, m], BF16, tag=f"g{q}")
                    nc.scalar.activation(a[:], g[:], FUNCS_B[q])
                    acts.append(a)
                gsl = lambda q: acts[q][:]
            return tail_bw(m, off, gsl, cp_ap, pairs, cnew_ap, out_l)

        def step_bw_lvl(lvl, cprev, out_l):
            """Generic bw level step (lvl >= 4): parents at lvl-1 cols."""
            m = 1 << (lvl - 3)
            off = _lvl_off(lvl)
            offp = _lvl_off(lvl - 1)
            cn = p_cbuf.tile([128, 4, m], BF16, tag="cbw")
            hf = None
            c0 = 0
            for mc in chunks_of(m):
                mpc = mc // 2
                hf = step_bw(
                    mc, off + c0,
                    lambda k, p0=offp + c0 // 2, mpc=mpc:
                        FB[4 + k][:, p0:p0 + mpc],
                    mpc, cprev[:, :, c0 // 2:c0 // 2 + mpc],
                    cn[:, :, c0:c0 + mc], out_l)
                c0 += mc
            return hf, cn

        # ------------------------------------------------------- scans
        def scans_emit(l, out_l):
            x2f, x2b = ST["x2f"], ST["x2b"]

            # fw lvl12 leaf (node 4095 slot, col 518)
            c12 = p_small.tile([128, 4, 1], BF16, tag="c12")
            h12 = leaf_fw(1, 518, c12[:], out_l)

            # bw top: nodes 0..6
            h_b0, cb0 = leaf_bw0(out_l)
            cb1 = p_cbuf.tile([128, 4, 2], BF16, tag="cbw")
            h_b1 = step_bw(2, 1, lambda k: FB[4 + k][:, 0:1], 1,
                           cb0[:, :, 0:1], cb1[:], out_l)
            cb2 = p_cbuf.tile([128, 4, 4], BF16, tag="cbw")
            h_b2 = step_bw(4, 3, lambda k: FB[4 + k][:, 1:3], 2,
                           cb1[:, :, 0:2], cb2[:], out_l)

            # bw lvl3 (subtree root, col 7): psel one-hot parent selection.
            # Emitted before the leaf floods so its DVE chain isn't queued
            # behind the leaf tails.
            hps = p_small.tile([128, 4, 1], BF16, tag="hps")
            cps = p_small.tile([128, 4, 1], F32, tag="cps")
            hsel = p_small.tile([128, 4, 1], F32, tag="hsel")
            for j in range(4):
                tsel = p_small.tile([128, 4], F32, tag="tsel")
                nc.vector.tensor_tensor(tsel[:], h_b2[:, j, :], psel_t[:],
                                        op=ALU.mult)
                nc.vector.tensor_reduce(hsel[:, j, :], tsel[:],
                                        mybir.AxisListType.X, ALU.add)
                tsel2 = p_small.tile([128, 4], F32, tag="tsel2")
                nc.vector.tensor_tensor(tsel2[:], cb2[:, j, :], psel_t[:],
                                        op=ALU.mult)
                nc.vector.tensor_reduce(cps[:, j, :], tsel2[:],
                                        mybir.AxisListType.X, ALU.add)
            nc.vector.tensor_copy(hps[:], hsel[:])
            cb3 = p_cbuf.tile([128, 4, 1], BF16, tag="cbw")
            step_bw(1, 7, lambda k: hps[:, k, :], 1, cps[:], cb3[:], out_l)

            # fw lvl11 leaf chunk 1 (cols 262..389) + col-262 correction
            c11 = p_cbuf.tile([128, 4, 256], BF16, tag="cfw")
            leaf_fw(128, 262, c11[:, :, 0:128], out_l, skip0=True)
            hlc = p_small.tile([128, 4, 1], BF16, tag="hlc")
            clc = p_small.tile([128, 4, 1], BF16, tag="clc")
            nc.vector.tensor_scalar(hlc[:], h12[:], lmask_t[:], None,
                                    op0=ALU.mult)
            nc.vector.tensor_scalar(clc[:], c12[:], lmask_t[:], None,
                                    op0=ALU.mult)
            step_fw(1, 262, lambda k: hlc[:, k, :], None, clc[:], None,
                    c11[:, :, 0:1], out_l, use_wr=False)

            # fw lvl11 leaf chunk 2
            leaf_fw(128, 390, c11[:, :, 128:256], out_l)

            st = {"cf": c11, "cb": cb3, "hro": None, "cro": None}

            def emit_bw(lvl):
                if lvl == 12:
                    cn = p_small.tile([128, 4, 1], BF16, tag="c12b")
                    step_bw(1, 518, lambda k: FB[4 + k][:, 262:263], 1,
                            st["cb"][:, :, 0:1], cn[:], out_l)
                    return
                hf, cn = step_bw_lvl(lvl, st["cb"], out_l)
                st["cb"] = cn

            def emit_fw(lvl):
                hf, cn = step_fw_lvl(lvl, st["cf"], out_l)
                st["hro"], st["cf"] = hf, cn

            for blvl, flvl in ((4, 10), (5, 9), (6, 8), (7, 7), (8, 6),
                               (9, 5), (10, 4), (11, 3)):
                emit_bw(blvl)
                emit_fw(flvl)
            emit_bw(12)
            return st["hro"], st["cf"]

        def fwtop_emit(hroots_bf, croots, out_l):
            ct2 = p_small.tile([128, 4, 4], BF16, tag="ct2")
            h2 = step_fw(4, 3,
                         lambda k: hroots_bf[:, k, 0:8:2],
                         lambda k: hroots_bf[:, k, 1:8:2],
                         croots[:, :, 0:8:2], croots[:, :, 1:8:2],
                         ct2[:], out_l)
            ct1 = p_small.tile([128, 4, 2], BF16, tag="ct1")
            h1 = step_fw(2, 1,
                         lambda k: FB[k][:, 3:7:2],
                         lambda k: FB[k][:, 4:7:2],
                         ct2[:, :, 0:4:2], ct2[:, :, 1:4:2],
                         ct1[:], out_l)
            ct0 = p_small.tile([128, 4, 1], BF16, tag="ct0")
            step_fw(1, 0,
                    lambda k: FB[k][:, 1:3:2],
                    lambda k: FB[k][:, 2:3:2],
                    ct1[:, :, 0:1], ct1[:, :, 1:2],
                    ct0[:], out_l)

        # =================================================== layer 0
        T0 = alloc_layer(0)
        ST.update(x2f=T0["x2f"], x2b=T0["x2b"])
        W0 = alloc_scan_weights(0)
        CUR.update(W0)
        proj_pass(0, T0, "b", [(0, 260), (260, 520)])
        proj_pass(0, T0, "f", [(0, 260), (260, 520)])
        # snapshot the fw head (cols 0..6) so fwtop doesn't pin the x2f buffer
        hd0 = p_small.tile([128, 28, 7], BF16, tag="hd0")
        nc.vector.tensor_copy(hd0[:], T0["x2f"][:, :, 0:7])
        hro0, cf0 = scans_emit(0, False)

        # allgather subtree roots of layer 0
        ccin = p_dram.tile([1024], F32, tag="ccin")
        ccout = p_dram.tile([NCORES, 1024], F32, tag="ccout")
        ccin_v = ccin[:].rearrange("(j p) -> p j", p=128)
        nc.gpsimd.dma_start(ccin_v[:, 0:4], hro0[:, :, 0])
        cfs = p_small.tile([128, 4], F32, tag="cfs")
        nc.vector.tensor_copy(cfs[:], cf0[:, :, 0])
        nc.gpsimd.dma_start(ccin_v[:, 4:8], cfs[:])
        nc.gpsimd.collective_compute(
            "AllGather", mybir.AluOpType.bypass,
            replica_groups=[list(range(NCORES))],
            ins=[ccin[:].opt()], outs=[ccout[:].opt()])
        ccout_v = ccout[:].rearrange("r (j p) -> p j r", p=128)
        hroots = p_small.tile([128, 4, 8], F32, tag="hroots")
        croots = p_small.tile([128, 4, 8], F32, tag="croots")
        for j in range(4):
            nc.gpsimd.dma_start(hroots[:, j, :], ccout_v[:, j, :])
            nc.gpsimd.dma_start(croots[:, j, :], ccout_v[:, 4 + j, :])
        hroots_bf = p_small.tile([128, 4, 8], BF16, tag="hrootsb")
        nc.vector.tensor_copy(hroots_bf[:], hroots[:])

        # =================================================== layer 1
        T1 = alloc_layer(1)
        ST.update(x2f=T1["x2f"], x2b=T1["x2b"])
        hfs = [p_hfs.tile([128, 134], F32, tag=f"hfs{j}", name=f"hfs{j}")
               for j in range(8)]
        ST["hfs"] = hfs

        # main passes (subtree cols; overlap the collective)
        proj_pass(1, T1, "b", [(7, 264), (264, 520)])
        proj_pass(1, T1, "f", [(7, 264), (264, 520)])

        # finish layer 0: redundant top-7 fw scan (layer-0 weights, head x2)
        ST.update(x2f=hd0)
        fwtop_emit(hroots_bf, croots, False)
        ST.update(x2f=T1["x2f"])

        # bw head pass (cols 0..6 need fwtop-0)
        proj_pass(1, T1, "b", [(0, 7)])
        for j in range(8):
            nc.sync.dma_start(f2top_d[j], FB[j][:, 0:7])

        W1 = alloc_scan_weights(1)
        CUR.update(W1)
        hro1, cf1 = scans_emit(1, True)
        rcs = p_small.tile([128, 4], F32, tag="rcs")
        nc.vector.tensor_copy(rcs[:], cf1[:, :, 0])
        nc.sync.dma_start(rootc_d[:], rcs[:])
        for j in range(8):
            nc.gpsimd.dma_start(outT_d[j][:, 0:134], hfs[j][:])

    nc.compile()
    return nc


# ------------------------------------------------------------------ packing
def _pack_inputs(inp):
    def bf(x):
        return np.ascontiguousarray(x).astype(BF16NP)

    feats = np.asarray(inp["features"], np.float32)
    per_core = []
    wl = np.stack([np.asarray(inp["fw_Wl"][l], np.float32)[_PF].T
                   .reshape(4, 128, 3072).transpose(1, 0, 2) for l in range(L)])
    wr = np.stack([np.asarray(inp["fw_Wr"][l], np.float32)[_PF].T
                   .reshape(4, 128, 3072).transpose(1, 0, 2) for l in range(L)])
    wh = np.stack([np.asarray(inp["bw_Wh"][l], np.float32)[_PB].T
                   .reshape(4, 128, 2560).transpose(1, 0, 2) for l in range(L)])

    def proj_pack(w):
        # w = W.T [1024, M] -> [M/128, 128p, 8k, 128m]
        M = w.shape[1]
        v = w.reshape(8, 128, M // 128, 128)  # (k, p, t, m)
        return np.ascontiguousarray(v.transpose(2, 1, 0, 3))

    wxf = np.stack([
        proj_pack(np.concatenate(
            [np.asarray(inp["fw_Wx"][l], np.float32)[_PF],
             np.asarray(inp["fw_Wp"][l], np.float32)], 0).T)
        for l in range(L)])
    wxb = np.stack([
        proj_pack(np.concatenate(
            [np.asarray(inp["bw_Wx"][l], np.float32)[_PB],
             np.asarray(inp["bw_Wp"][l], np.float32)], 0).T)
        for l in range(L)])
    bfv = np.stack([
        np.concatenate([
            (np.asarray(inp["fw_bx"][l], np.float32)
             + np.asarray(inp["fw_bl"][l], np.float32)
             + np.asarray(inp["fw_br"][l], np.float32))[_PF],
            np.asarray(inp["fw_bp"][l], np.float32)], 0)
        .reshape(28, 128).T for l in range(L)])
    bbv = np.stack([
        np.concatenate([
            (np.asarray(inp["bw_bx"][l], np.float32)
             + np.asarray(inp["bw_bh"][l], np.float32))[_PB],
            np.asarray(inp["bw_bp"][l], np.float32)], 0)
        .reshape(24, 128).T for l in range(L)])
    base = {
        "wl": bf(wl), "wr": bf(wr), "wh": bf(wh),
        "wxf": bf(wxf), "wxb": bf(wxb),
        "bf": np.ascontiguousarray(bfv, dtype=np.float32),
        "bb": np.ascontiguousarray(bbv, dtype=np.float32),
    }
    for c in range(NCORES):
        cm = _col_map(c)
        v = cm >= 0
        fT = np.zeros((1024, NC), np.float32)
        fT[:, v] = feats[cm[v]].T
        psel = np.zeros((128, 4), np.float32)
        psel[:, c // 2] = 1.0
        lmask = np.full((128, 1), 1.0 if c == 0 else 0.0, np.float32)
        m = dict(base)
        m["featsT"] = bf(fT.reshape(8, 128, NC))
        m["psel"] = psel
        m["lmask"] = lmask
        per_core.append(m)
    return per_core


def _host_fwtop(inp, results):
    """Compute the final layer's top-7 forward h on the host, mirroring the
    device arithmetic (bf16 matmul inputs, fp32 accumulation)."""
    l = L - 1

    def bf(x):
        return x.astype(BF16NP).astype(np.float32)

    def sig(x):
        return 1.0 / (1.0 + np.exp(-x))

    # features of layer 1 at nodes 0..6 (bf16 as on device)
    f2 = np.concatenate([np.asarray(results[0]["f2top"], np.float32)[j]
                         for j in range(8)], 0)  # [1024, 7]
    wxf = np.concatenate([np.asarray(inp["fw_Wx"][l], np.float32),
                          np.asarray(inp["fw_Wp"][l], np.float32)], 0)
    bxf = np.concatenate([
        np.asarray(inp["fw_bx"][l], np.float32)
        + np.asarray(inp["fw_bl"][l], np.float32)
        + np.asarray(inp["fw_br"][l], np.float32),
        np.asarray(inp["fw_bp"][l], np.float32)], 0)
    pf = bf(wxf) @ f2 + bxf[:, None]  # f2 already bf16-rounded
    x2, px = pf[:3072], pf[3072:]
    wl = bf(np.asarray(inp["fw_Wl"][l], np.float32))
    wr = bf(np.asarray(inp["fw_Wr"][l], np.float32))

    # subtree roots: h from outT col 7 (f32), c from rootc
    hr8 = np.stack([np.asarray(results[c]["outT"], np.float32)
                    .reshape(1024, NC)[0:512, 7] for c in range(NCORES)], 1)
    cr8 = np.stack([np.asarray(results[c]["rootc"], np.float32)
                    .T.reshape(512) for c in range(NCORES)], 1)

    hbuf = np.zeros((512, 7), np.float32)
    cbuf = np.zeros((512, 7), np.float32)

    def step(cols, hl, hr, cl, cr):
        g = x2[:, cols] + wl @ bf(hl) + wr @ bf(hr)
        i_, o, fl, fr, u, r = (g[k * 512:(k + 1) * 512] for k in range(6))
        i_, o, fl, fr, r = sig(i_), sig(o), sig(fl), sig(fr), sig(r)
        u = np.tanh(u)
        cc = i_ * u + fl * cl + fr * cr
        hc = o * np.tanh(cc)
        return cc, hc * r + (1.0 - r) * px[:, cols]

    cc, hf = step([3, 4, 5, 6], hr8[:, 0::2], hr8[:, 1::2],
                  cr8[:, 0::2], cr8[:, 1::2])
    hbuf[:, 3:7], cbuf[:, 3:7] = hf, cc
    cc, hf = step([1, 2], hbuf[:, 3:7:2], hbuf[:, 4:7:2],
                  cbuf[:, 3:7:2], cbuf[:, 4:7:2])
    hbuf[:, 1:3], cbuf[:, 1:3] = hf, cc
    cc, hf = step([0], hbuf[:, 1:2], hbuf[:, 2:3],
                  cbuf[:, 1:2], cbuf[:, 2:3])
    hbuf[:, 0:1] = hf
    return hbuf  # [512, 7]


def _assemble(inp, results):
    out = np.zeros((N, 1024), np.float32)
    for c in range(NCORES):
        cm = _col_map(c)
        o = np.asarray(results[c]["outT"]).reshape(1024, NC)
        cols = np.arange(NC)
        use = (cm >= 0) & (cols >= 7) & (cols != 519)
        if c != 0:
            use &= cols != 518
        out[cm[use]] = o[:, use].T
        if c == 0:
            out[0:7, 512:1024] = o[512:1024, 0:7].T  # bw half from device
    out[0:7, 0:512] = _host_fwtop(inp, results).T
    return out


def kernel(**inputs):
    inp = {k: np.asarray(v) for k, v in inputs.items()}
    if not _is_canonical(inp):
        return _fallback(inp)
    if "nc" not in _CACHE:
        _CACHE["nc"] = _build_nc()
    from concourse.bass_utils import run_bass_kernel_spmd

    in_maps = _pack_inputs(inp)
    res = run_bass_kernel_spmd(_CACHE["nc"], in_maps,
                               core_ids=list(range(NCORES)))
    return _assemble(inp, res.results)


if __name__ == "__main__":
    d = np.load("/tmp/inputs.npz")
    inputs = {k: d[k] for k in d.files}
    expected = np.load("/tmp/expected.npy")
    actual = kernel(**inputs)
    err = np.abs(actual - expected)
    print("max abs err:", err.max())
    print("absmax-rel:", err.max() / np.abs(expected).max())
    print("mean abs:", err.mean())
